# revision 17
# baseline (speedup 1.0000x reference)
"""2-layer GAT + global mean pool + linear + softmax on 8 Trainium2 cores.

Self-contained Bass/Tile kernel. Sharding: dst-nodes uniformly across the 8
cores; edges sorted by dst; every 128-dst block padded to a uniform tile
count so one SPMD instruction stream serves all cores. h1 is AllGather'ed
between the layers; pooled partials are AllReduce'd at the end.
"""

import dataclasses
import sys

import numpy as np

for _p in ("/opt/trn_rl_repo", "/opt/trn_rl_repo/concourse"):
    if _p not in sys.path:
        sys.path.insert(0, _p)

import concourse.bass as bass
import concourse.bacc as bacc
import concourse.mybir as mybir
import concourse.tile as tile
from concourse import bass_utils
from concourse.masks import make_identity

F32 = mybir.dt.float32
BF16 = mybir.dt.bfloat16
I32 = mybir.dt.int32
U8 = mybir.dt.uint8

HEADS, HID, FEAT, OUT = 4, 32, 128, 10
ROWB = 272  # H' row bytes: 128 bf16 + 4 f32 (al_src)
NEG_SLOPE = 0.2
EPS = 1e-16


@dataclasses.dataclass
class Cfg:
    ncores: int
    n: int
    g: int
    nb: int          # dst blocks per core
    tb: int          # tiles (128 edges) per block — uniform across cores
    xt_grp: int = 8
    h_grp: int = 4

    @property
    def npad(self):
        return ((self.n + 127) // 128) * 128

    @property
    def sh(self):
        return self.nb * 128

    @property
    def n2(self):
        return self.ncores * self.sh

    @property
    def nt(self):
        return self.nb * self.tb

    @property
    def ne(self):
        return self.nt * 128


def host_prep(x, W1, a1_src, a1_dst, b1, W2, a2_src, a2_dst, b2, Wl, bl,
              edge_index, batch, ncores):
    n = x.shape[0]
    g = 64
    e0 = np.asarray(edge_index[0], np.int64)
    e1 = np.asarray(edge_index[1], np.int64)
    batch = np.asarray(batch, np.int64)
    loops = np.arange(n, dtype=np.int64)
    src = np.concatenate([e0, loops])
    dst = np.concatenate([e1, loops])
    order = np.argsort(dst, kind="stable")
    src, dst = src[order], dst[order]

    assert n % ncores == 0, (n, ncores)
    per = n // ncores
    nb = (per + 127) // 128

    counts = np.zeros((ncores, nb), np.int64)
    core_of = dst // per
    loc = dst - core_of * per
    blk = loc // 128
    np.add.at(counts, (core_of, blk), 1)
    tb = int(np.ceil(counts.max() / 128))
    cfg = Cfg(ncores=ncores, n=n, g=g, nb=nb, tb=tb)

    cnt_g = np.bincount(batch, minlength=g).astype(np.float64)
    wg = (1.0 / np.maximum(cnt_g, 1.0)).astype(np.float32)

    node_core = np.arange(n) // per
    node_loc = np.arange(n) - node_core * per
    remap = node_core * cfg.sh + node_loc

    ne = cfg.ne
    core_lo = np.searchsorted(dst, np.arange(ncores) * per)
    core_hi = np.searchsorted(dst, (np.arange(ncores) + 1) * per)

    b1m = np.broadcast_to(np.asarray(b1, np.float32), (128, FEAT)).copy()
    b2m = np.broadcast_to(np.asarray(b2, np.float32), (128, FEAT)).copy()

    def wcat(W, a_s, a_d):
        As = np.zeros((FEAT, HEADS), np.float32)
        Ad = np.zeros((FEAT, HEADS), np.float32)
        for h in range(HEADS):
            As[h * HID:(h + 1) * HID, h] = a_s[h]
            Ad[h * HID:(h + 1) * HID, h] = a_d[h]
        return np.concatenate([np.asarray(W, np.float32), As, Ad], 1)

    w1c = wcat(W1, a1_src, a1_dst)
    w2c = wcat(W2, a2_src, a2_dst)
    xt = np.zeros((FEAT, cfg.npad), np.float32)
    xt[:, :n] = np.asarray(x, np.float32).T

    in_maps = []
    for k in range(ncores):
        s, e = core_lo[k], core_hi[k]
        csrc, cdst = src[s:e], dst[s:e]
        cloc = cdst - k * per
        cblk = cloc // 128
        S1 = np.zeros(ne, np.int32)
        D1 = np.zeros(ne, np.int32)
        S2 = np.zeros(ne, np.int32)
        D2 = np.zeros(ne, np.int32)
        DL = np.full(ne, -1.0, np.float32)
        bs = np.zeros(nb + 1, np.int64)
        np.add.at(bs[1:], cblk, 1)
        offs = np.cumsum(bs)[:-1]
        pos = (cblk * cfg.tb * 128) + (np.arange(len(csrc)) - offs[cblk])
        S1[pos] = csrc
        D1[pos] = cdst
        S2[pos] = remap[csrc]
        D2[pos] = remap[cdst]
        DL[pos] = (cloc % 128).astype(np.float32)

        def tposed(a):
            return np.ascontiguousarray(a.reshape(cfg.nt, 128).T)

        pid = np.full(cfg.sh, -1.0, np.float32)
        pw = np.zeros(cfg.sh, np.float32)
        lo, hi = k * per, (k + 1) * per
        pid[:per] = batch[lo:hi].astype(np.float32)
        pw[:per] = wg[batch[lo:hi]]
        in_maps.append({
            "XT": xt, "W1cat": w1c, "W2cat": w2c,
            "B1M": b1m, "B2M": b2m,
            "WL": np.asarray(Wl, np.float32),
            "BLC": np.asarray(bl, np.float32).reshape(OUT, 1),
            "SRC1": tposed(S1), "DST1": tposed(D1),
            "SRC2": tposed(S2), "DST2": tposed(D2),
            "DLOC": tposed(DL.view(np.int32)),
            "PID": np.ascontiguousarray(pid.reshape(nb, 128).T),
            "PW": np.ascontiguousarray(pw.reshape(nb, 128).T),
        })
    return cfg, in_maps


def build(cfg: Cfg):
    nc = bacc.Bacc("TRN2", target_bir_lowering=False, debug=False,
                   num_devices=cfg.ncores)
    NB, TB, NT = cfg.nb, cfg.tb, cfg.nt
    NP1 = cfg.npad // 128
    NP2 = cfg.n2 // 128

    ein = lambda nm, sh, dt: nc.dram_tensor(nm, sh, dt, kind="ExternalInput").ap()
    XT = ein("XT", [FEAT, cfg.npad], F32)
    W1c = ein("W1cat", [FEAT, 136], F32)
    W2c = ein("W2cat", [FEAT, 136], F32)
    B1M = ein("B1M", [128, FEAT], F32)
    B2M = ein("B2M", [128, FEAT], F32)
    WL = ein("WL", [FEAT, OUT], F32)
    BLC = ein("BLC", [OUT, 1], F32)
    SRC1 = ein("SRC1", [128, NT], I32)
    DST1 = ein("DST1", [128, NT], I32)
    SRC2 = ein("SRC2", [128, NT], I32)
    DST2 = ein("DST2", [128, NT], I32)
    DLOC = ein("DLOC", [128, NT], I32)
    PID = ein("PID", [128, NB], F32)
    PW = ein("PW", [128, NB], F32)
    OUTT = nc.dram_tensor("OUT", [64, OUT], F32, kind="ExternalOutput").ap()

    H1p = nc.dram_tensor("H1p", [cfg.npad, ROWB], U8).ap()
    ALd1 = nc.dram_tensor("ALd1", [cfg.npad, HEADS], F32).ap()
    h1sh = nc.dram_tensor("h1sh", [cfg.sh, FEAT], BF16).ap()
    h1full = nc.dram_tensor("h1full", [cfg.n2, FEAT], BF16,
                            addr_space="Shared").ap()
    H2p = nc.dram_tensor("H2p", [cfg.n2, ROWB], U8).ap()
    ALd2 = nc.dram_tensor("ALd2", [cfg.n2, HEADS], F32).ap()
    prt = nc.dram_tensor("prt", [FEAT, 64], F32).ap()
    prf = nc.dram_tensor("prf", [FEAT, 64], F32, addr_space="Shared").ap()

    groups = [list(range(cfg.ncores))]

    with tile.TileContext(nc) as tc:
        import contextlib
        ctx = contextlib.ExitStack()
        with ctx:
            consts = ctx.enter_context(tc.tile_pool(name="consts", bufs=1))
            xtp = ctx.enter_context(tc.tile_pool(name="xtp", bufs=3))
            hp = ctx.enter_context(tc.tile_pool(name="hp", bufs=3))
            hps = ctx.enter_context(tc.tile_pool(name="hps", bufs=2, space="PSUM"))
            gp = ctx.enter_context(tc.tile_pool(name="gp", bufs=4))
            sp = ctx.enter_context(tc.tile_pool(name="sp", bufs=4))
            up = ctx.enter_context(tc.tile_pool(name="up", bufs=3, space="PSUM"))
            ep = ctx.enter_context(tc.tile_pool(name="ep", bufs=4))
            pp = ctx.enter_context(tc.tile_pool(name="pp", bufs=1, space="PSUM"))
            tp = ctx.enter_context(tc.tile_pool(name="tp", bufs=1))

            w1sb = consts.tile([FEAT, 136], F32, tag="w1")
            nc.sync.dma_start(out=w1sb[:], in_=W1c)
            w2sb_f = consts.tile([FEAT, 136], F32, tag="w2f")
            nc.sync.dma_start(out=w2sb_f[:], in_=W2c)
            w2sb = consts.tile([FEAT, 136], BF16, tag="w2b")
            nc.vector.tensor_copy(out=w2sb[:], in_=w2sb_f[:])
            b1sb = consts.tile([128, FEAT], F32, tag="b1")
            nc.sync.dma_start(out=b1sb[:], in_=B1M)
            b2sb = consts.tile([128, FEAT], F32, tag="b2")
            nc.sync.dma_start(out=b2sb[:], in_=B2M)
            wlsb = consts.tile([FEAT, OUT], F32, tag="wl")
            nc.sync.dma_start(out=wlsb[:], in_=WL)
            blsb = consts.tile([OUT, 1], F32, tag="bl")
            nc.sync.dma_start(out=blsb[:], in_=BLC)
            ident = consts.tile([128, 128], F32, tag="ident")
            make_identity(nc, ident[:])
            iotaI = consts.tile([128, 128], I32, tag="iotai")
            nc.gpsimd.iota(iotaI[:], pattern=[[1, 128]], base=0,
                           channel_multiplier=0)
            iotaB = consts.tile([128, 128], BF16, tag="iotab")
            nc.vector.tensor_copy(out=iotaB[:], in_=iotaI[:])
            iota64 = consts.tile([128, 64], F32, tag="iota64")
            nc.vector.tensor_copy(out=iota64[:], in_=iotaI[:, :64])
            # dense [0..127] repeated TB times — dense src for the S build
            iotaR = consts.tile([128, TB * 128], BF16, tag="iotar")
            for t in range(TB):
                nc.vector.tensor_copy(out=iotaR[:, t * 128:(t + 1) * 128],
                                      in_=iotaB[:])
            pidsb = consts.tile([128, NB], F32, tag="pid")
            nc.sync.dma_start(out=pidsb[:], in_=PID)
            pwsb = consts.tile([128, NB], F32, tag="pw")
            nc.sync.dma_start(out=pwsb[:], in_=PW)

            idxp = ctx.enter_context(tc.tile_pool(name="idxp", bufs=1))
            src1 = idxp.tile([128, NT], I32, tag="src1")
            nc.sync.dma_start(out=src1[:], in_=SRC1)
            dst1 = idxp.tile([128, NT], I32, tag="dst1")
            nc.sync.dma_start(out=dst1[:], in_=DST1)
            src2 = idxp.tile([128, NT], I32, tag="src2")
            nc.sync.dma_start(out=src2[:], in_=SRC2)
            dst2 = idxp.tile([128, NT], I32, tag="dst2")
            nc.sync.dma_start(out=dst2[:], in_=DST2)
            dlocI = idxp.tile([128, NT], I32, tag="dloc")
            nc.sync.dma_start(out=dlocI[:], in_=DLOC)
            dlocF = dlocI[:].bitcast(F32)
            dlocB = idxp.tile([128, NT], BF16, tag="dlocb")
            nc.vector.tensor_copy(out=dlocB[:], in_=dlocF)

            def h_phase(nblocks, lhs_group, wsb, Hp, ALd):
                grp = cfg.xt_grp
                for m0 in range(0, nblocks, grp):
                    mcnt = min(grp, nblocks - m0)
                    lhs_of = lhs_group(m0, mcnt)
                    sb = hp.tile([128, grp * ROWB], U8, tag="hrow")
                    sbB = sb[:].bitcast(BF16)
                    sbF = sb[:].bitcast(F32)
                    asb = hp.tile([128, grp * HEADS], F32, tag="ald")
                    for j in range(mcnt):
                        m = m0 + j
                        ps = hps.tile([128, 136], F32, tag="hpsum")
                        nc.tensor.matmul(ps[:], lhsT=lhs_of(j), rhs=wsb[:],
                                         start=True, stop=True)
                        if j % 2 == 0:
                            nc.scalar.copy(out=sbB[:, j * 136:j * 136 + FEAT],
                                           in_=ps[:, :FEAT])
                        else:
                            nc.vector.tensor_copy(
                                out=sbB[:, j * 136:j * 136 + FEAT],
                                in_=ps[:, :FEAT])
                        nc.vector.tensor_copy(
                            out=sbF[:, j * 68 + 64:j * 68 + 68],
                            in_=ps[:, FEAT:FEAT + HEADS])
                        nc.vector.tensor_copy(
                            out=asb[:, j * HEADS:(j + 1) * HEADS],
                            in_=ps[:, FEAT + HEADS:FEAT + 2 * HEADS])
                    ho = Hp[m0 * 128:(m0 + mcnt) * 128, :].rearrange(
                        "(b p) c -> p b c", p=128)
                    nc.scalar.dma_start(
                        out=ho, in_=sb[:, :mcnt * ROWB].rearrange(
                            "p (b c) -> p b c", c=ROWB))
                    ao = ALd[m0 * 128:(m0 + mcnt) * 128, :].rearrange(
                        "(b p) c -> p b c", p=128)
                    nc.scalar.dma_start(
                        out=ao, in_=asb[:, :mcnt * HEADS].rearrange(
                            "p (b c) -> p b c", c=HEADS))

            def p1_lhs(m0, mcnt):
                t = xtp.tile([128, cfg.xt_grp * 128], F32, tag="xt")
                nc.sync.dma_start(out=t[:, :mcnt * 128],
                                  in_=XT[:, m0 * 128:(m0 + mcnt) * 128])
                return lambda j: t[:, j * 128:(j + 1) * 128]

            h_phase(NP1, p1_lhs, w1sb, H1p, ALd1)

            def edge_phase(srcT, dstT, Hp, ALd, layer):
                hgrp = cfg.h_grp
                h1grp = None
                g1 = g2 = None
                PAIR = 1  # blocks per gather instruction
                for b in range(NB):
                    if b % PAIR == 0:
                        bw = min(PAIR, NB - b)  # blocks in this gather
                        t0g = b * TB
                        g1 = gp.tile([128, PAIR * TB * ROWB], U8, tag="g1")
                        nc.gpsimd.indirect_dma_start(
                            out=g1[:, :bw * TB * ROWB], out_offset=None,
                            in_=Hp,
                            in_offset=bass.IndirectOffsetOnAxis(
                                ap=srcT[:, t0g:t0g + bw * TB], axis=0))
                        g2 = gp.tile([128, PAIR * TB * HEADS], F32, tag="g2")
                        nc.gpsimd.indirect_dma_start(
                            out=g2[:, :bw * TB * HEADS], out_offset=None,
                            in_=ALd,
                            in_offset=bass.IndirectOffsetOnAxis(
                                ap=dstT[:, t0g:t0g + bw * TB], axis=0))
                    sub = b % PAIR
                    t0 = b * TB
                    g1B = g1[:].bitcast(BF16)[
                        :, sub * TB * 136:(sub + 1) * TB * 136]
                    g1F = g1[:].bitcast(F32)[
                        :, sub * TB * 68:(sub + 1) * TB * 68]
                    g2s = g2[:, sub * TB * HEADS:(sub + 1) * TB * HEADS]
                    lg = sp.tile([128, TB * HEADS], F32, tag="lg")
                    als = g1F.rearrange("p (t c) -> p t c", c=68)[:, :, 64:68]
                    nc.vector.tensor_tensor(
                        out=lg[:].rearrange("p (t c) -> p t c", c=HEADS),
                        in0=als,
                        in1=g2s.rearrange("p (t c) -> p t c", c=HEADS),
                        op=mybir.AluOpType.add)
                    lg2 = sp.tile([128, TB * HEADS], F32, tag="lg2")
                    nc.vector.tensor_scalar_mul(lg2[:], lg[:], NEG_SLOPE)
                    nc.vector.tensor_tensor(out=lg[:], in0=lg[:], in1=lg2[:],
                                            op=mybir.AluOpType.max)
                    pv = g1B.rearrange("p (t c) -> p t c", c=136)[
                        :, :, FEAT:FEAT + HEADS]
                    nc.scalar.activation(
                        out=pv,
                        in_=lg[:].rearrange("p (t c) -> p t c", c=HEADS),
                        func=mybir.ActivationFunctionType.Exp)
                    # dense broadcast expansions on ScalarE so the two big
                    # VectorE tensor_tensor ops run in the 2x perf mode
                    dlocX = sp.tile([128, TB * 128], BF16, tag="dlocX")
                    nc.scalar.copy(
                        out=dlocX[:].rearrange("p (t c) -> p t c", c=128),
                        in_=dlocB[:, t0:t0 + TB].unsqueeze(2).to_broadcast(
                            [128, TB, 128]))
                    pX = sp.tile([128, TB * 128], BF16, tag="pX")
                    nc.scalar.copy(
                        out=pX[:].rearrange("p (t h c) -> p t h c",
                                            h=HEADS, c=HID),
                        in_=g1B.rearrange("p (t c) -> p t c", c=136)[
                            :, :, FEAT:FEAT + HEADS].unsqueeze(3).to_broadcast(
                            [128, TB, HEADS, HID]))
                    S = sp.tile([128, TB * 128], BF16, tag="S")
                    nc.vector.tensor_tensor(
                        out=S[:], in0=iotaR[:], in1=dlocX[:],
                        op=mybir.AluOpType.is_equal)
                    hv = g1B.rearrange("p (t c) -> p t c", c=136)[:, :, :FEAT]
                    nc.vector.tensor_tensor(
                        out=hv, in0=hv,
                        in1=pX[:].rearrange("p (t c) -> p t c", c=128),
                        op=mybir.AluOpType.mult)
                    ups = up.tile([128, FEAT + HEADS], F32, tag="u")
                    for t in range(TB):
                        nc.tensor.matmul(
                            ups[:],
                            lhsT=S[:, t * 128:(t + 1) * 128],
                            rhs=g1B[:, t * 136:t * 136 + FEAT + HEADS],
                            start=(t == 0), stop=(t == TB - 1))
                    z = ep.tile([128, HEADS], F32, tag="z")
                    nc.vector.tensor_scalar_add(z[:], ups[:, FEAT:FEAT + HEADS],
                                                EPS)
                    rz = ep.tile([128, HEADS], F32, tag="rz")
                    nc.vector.reciprocal(rz[:], z[:])
                    o1 = ep.tile([128, FEAT], F32, tag="o1")
                    nc.vector.tensor_tensor(
                        out=o1[:].rearrange("p (h c) -> p h c", c=HID),
                        in0=ups[:, :FEAT].rearrange("p (h c) -> p h c", c=HID),
                        in1=rz[:].unsqueeze(2).to_broadcast([128, HEADS, HID]),
                        op=mybir.AluOpType.mult)
                    if layer == 1:
                        nc.vector.tensor_tensor(out=o1[:], in0=o1[:],
                                                in1=b1sb[:],
                                                op=mybir.AluOpType.add)
                        mn = ep.tile([128, FEAT], F32, tag="mn")
                        nc.vector.tensor_scalar_min(mn[:], o1[:], 0.0)
                        ex = ep.tile([128, FEAT], F32, tag="ex")
                        nc.scalar.activation(
                            out=ex[:], in_=mn[:],
                            func=mybir.ActivationFunctionType.Exp)
                        nc.vector.tensor_scalar(
                            o1[:], o1[:], 0.0, -1.0,
                            op0=mybir.AluOpType.max, op1=mybir.AluOpType.add)
                        if h1grp is None or b % hgrp == 0:
                            h1grp = ep.tile([128, hgrp * FEAT], BF16, tag="h1g")
                        nc.vector.tensor_tensor(
                            out=h1grp[:, (b % hgrp) * FEAT:
                                      (b % hgrp + 1) * FEAT],
                            in0=o1[:], in1=ex[:], op=mybir.AluOpType.add)
                        if b % hgrp == hgrp - 1 or b == NB - 1:
                            blo = (b // hgrp) * hgrp
                            bcnt = b - blo + 1
                            ho = h1sh[blo * 128:(b + 1) * 128, :].rearrange(
                                "(q p) c -> p q c", p=128)
                            nc.scalar.dma_start(
                                out=ho,
                                in_=h1grp[:, :bcnt * FEAT].rearrange(
                                    "p (q c) -> p q c", c=FEAT))
                    else:
                        h2 = ep.tile([128, FEAT], F32, tag="h2")
                        nc.vector.tensor_tensor(out=h2[:], in0=o1[:],
                                                in1=b2sb[:],
                                                op=mybir.AluOpType.add)
                        spg = ep.tile([128, 64], F32, tag="spg")
                        nc.vector.tensor_tensor(
                            out=spg[:],
                            in0=pidsb[:, b:b + 1].to_broadcast([128, 64]),
                            in1=iota64[:], op=mybir.AluOpType.is_equal)
                        nc.vector.tensor_scalar(
                            spg[:], spg[:], pwsb[:, b:b + 1], None,
                            op0=mybir.AluOpType.mult)
                        nc.tensor.matmul(
                            ppsum[:], lhsT=h2[:], rhs=spg[:],
                            start=(b == 0), stop=(b == NB - 1),
                            skip_group_check=True)

            edge_phase(src1, dst1, H1p, ALd1, layer=1)

            nc.gpsimd.collective_compute(
                "AllGather", mybir.AluOpType.bypass, replica_groups=groups,
                ins=[h1sh.opt()], outs=[h1full.opt()])

            def p3_lhs(m0, mcnt):
                t = xtp.tile([128, cfg.xt_grp * 128], BF16, tag="h1t")
                nc.sync.dma_start(
                    out=t[:, :mcnt * 128],
                    in_=h1full[m0 * 128:(m0 + mcnt) * 128, :],
                    transpose=True)
                return lambda j: t[:, j * 128:(j + 1) * 128]

            h_phase(NP2, p3_lhs, w2sb, H2p, ALd2)

            ppsum = pp.tile([128, 64], F32, tag="pool")
            edge_phase(src2, dst2, H2p, ALd2, layer=2)

            psb = tp.tile([128, 64], F32, tag="psb")
            nc.vector.tensor_copy(out=psb[:], in_=ppsum[:])
            nc.sync.dma_start(out=prt, in_=psb[:])
            nc.gpsimd.collective_compute(
                "AllReduce", mybir.AluOpType.add, replica_groups=groups,
                ins=[prt.opt()], outs=[prf.opt()])
            pall = tp.tile([128, 64], F32, tag="pall")
            nc.sync.dma_start(out=pall[:], in_=prf)
            lps = up.tile([OUT, 64], F32, tag="u")
            nc.tensor.matmul(lps[:], lhsT=wlsb[:], rhs=pall[:],
                             start=True, stop=True)
            lsb = tp.tile([OUT, 64], F32, tag="lsb")
            nc.vector.tensor_scalar(lsb[:], lps[:], blsb[:, :1], None,
                                    op0=mybir.AluOpType.add)
            tps = up.tile([64, OUT], F32, tag="u")
            nc.tensor.transpose(out=tps[:], in_=lsb[:],
                                identity=ident[:OUT, :OUT])
            sm = tp.tile([64, OUT], F32, tag="sm")
            nc.vector.tensor_copy(out=sm[:], in_=tps[:])
            mx = tp.tile([64, 1], F32, tag="mx")
            nc.vector.reduce_max(mx[:], sm[:], axis=mybir.AxisListType.X)
            nc.vector.tensor_scalar(sm[:], sm[:], mx[:, :1], None,
                                    op0=mybir.AluOpType.subtract)
            nc.scalar.activation(out=sm[:], in_=sm[:],
                                 func=mybir.ActivationFunctionType.Exp)
            ssum = tp.tile([64, 1], F32, tag="ssum")
            nc.vector.reduce_sum(ssum[:], sm[:], axis=mybir.AxisListType.X)
            rs = tp.tile([64, 1], F32, tag="rs")
            nc.vector.reciprocal(rs[:], ssum[:])
            nc.vector.tensor_scalar(sm[:], sm[:], rs[:, :1], None,
                                    op0=mybir.AluOpType.mult)
            nc.sync.dma_start(out=OUTT, in_=sm[:])

    nc.compile()
    return nc


_CACHE = {}


def kernel(**inputs) -> np.ndarray:
    ncores = 8
    cfg, in_maps = host_prep(ncores=ncores, **inputs)
    key = dataclasses.astuple(cfg)
    if key not in _CACHE:
        _CACHE[key] = build(cfg)
    nc = _CACHE[key]
    res = bass_utils.run_bass_kernel_spmd(nc, in_maps,
                                          core_ids=list(range(ncores)))
    out = res.results[0]["OUT"][:64]
    return np.asarray(out, np.float32)


if __name__ == "__main__":
    # quick self-run with random data matching the spec
    rng = np.random.default_rng(0)
    ins = {
        "x": rng.standard_normal((50000, 128), np.float32),
        "W1": (rng.standard_normal((128, 128)) * 0.05).astype(np.float32),
        "a1_src": (rng.standard_normal((4, 32)) * 0.05).astype(np.float32),
        "a1_dst": (rng.standard_normal((4, 32)) * 0.05).astype(np.float32),
        "b1": np.zeros(128, np.float32),
        "W2": (rng.standard_normal((128, 128)) * 0.05).astype(np.float32),
        "a2_src": (rng.standard_normal((4, 32)) * 0.05).astype(np.float32),
        "a2_dst": (rng.standard_normal((4, 32)) * 0.05).astype(np.float32),
        "b2": np.zeros(128, np.float32),
        "Wl": (rng.standard_normal((128, 10)) * 0.05).astype(np.float32),
        "bl": np.zeros(10, np.float32),
        "edge_index": rng.integers(0, 50000, (2, 800000)).astype(np.int32),
        "batch": np.sort(rng.integers(0, 64, 50000)).astype(np.int32),
    }
    out = kernel(**ins)
    print(out.shape, out.dtype, out[:2])


# revision 18
# speedup vs baseline: 1.5810x; 1.5810x over previous
"""2-layer GAT + global mean pool + linear + softmax on 8 Trainium2 cores.

Self-contained Bass/Tile kernel. Sharding: dst-nodes uniformly across the 8
cores; edges sorted by dst; every 128-dst block padded to a uniform tile
count so one SPMD instruction stream serves all cores. h1 is AllGather'ed
between the layers; pooled partials are AllReduce'd at the end.
"""

import dataclasses
import sys

import numpy as np

for _p in ("/opt/trn_rl_repo", "/opt/trn_rl_repo/concourse"):
    if _p not in sys.path:
        sys.path.insert(0, _p)

import concourse.bass as bass
import concourse.bacc as bacc
import concourse.mybir as mybir
import concourse.tile as tile
from concourse import bass_utils
from concourse.masks import make_identity

F32 = mybir.dt.float32
BF16 = mybir.dt.bfloat16
I32 = mybir.dt.int32
U8 = mybir.dt.uint8

HEADS, HID, FEAT, OUT = 4, 32, 128, 10
ROWB = 288  # H' row bytes: 128 bf16 | 4 f32 al_src | 4 f32 al_dst
NEG_SLOPE = 0.2
EPS = 1e-16


@dataclasses.dataclass
class Cfg:
    ncores: int
    n: int
    g: int
    nb: int          # dst blocks per core
    tb: int          # tiles (128 edges) per block — uniform across cores
    xt_grp: int = 8
    h_grp: int = 4

    @property
    def npad(self):
        return ((self.n + 127) // 128) * 128

    @property
    def sh(self):
        return self.nb * 128

    @property
    def n2(self):
        return self.ncores * self.sh

    @property
    def nt(self):
        return self.nb * self.tb

    @property
    def ne(self):
        return self.nt * 128


def host_prep(x, W1, a1_src, a1_dst, b1, W2, a2_src, a2_dst, b2, Wl, bl,
              edge_index, batch, ncores):
    n = x.shape[0]
    g = 64
    e0 = np.asarray(edge_index[0], np.int64)
    e1 = np.asarray(edge_index[1], np.int64)
    batch = np.asarray(batch, np.int64)
    loops = np.arange(n, dtype=np.int64)
    src = np.concatenate([e0, loops])
    dst = np.concatenate([e1, loops])
    order = np.argsort(dst, kind="stable")
    src, dst = src[order], dst[order]

    assert n % ncores == 0, (n, ncores)
    per = n // ncores
    nb = (per + 127) // 128

    counts = np.zeros((ncores, nb), np.int64)
    core_of = dst // per
    loc = dst - core_of * per
    blk = loc // 128
    np.add.at(counts, (core_of, blk), 1)
    tb = int(np.ceil(counts.max() / 128))
    cfg = Cfg(ncores=ncores, n=n, g=g, nb=nb, tb=tb)

    cnt_g = np.bincount(batch, minlength=g).astype(np.float64)
    wg = (1.0 / np.maximum(cnt_g, 1.0)).astype(np.float32)

    node_core = np.arange(n) // per
    node_loc = np.arange(n) - node_core * per
    remap = node_core * cfg.sh + node_loc

    ne = cfg.ne
    core_lo = np.searchsorted(dst, np.arange(ncores) * per)
    core_hi = np.searchsorted(dst, (np.arange(ncores) + 1) * per)

    b1m = np.broadcast_to(np.asarray(b1, np.float32), (128, FEAT)).copy()
    b2m = np.broadcast_to(np.asarray(b2, np.float32), (128, FEAT)).copy()

    def wcat(W, a_s, a_d):
        As = np.zeros((FEAT, HEADS), np.float32)
        Ad = np.zeros((FEAT, HEADS), np.float32)
        for h in range(HEADS):
            As[h * HID:(h + 1) * HID, h] = a_s[h]
            Ad[h * HID:(h + 1) * HID, h] = a_d[h]
        return np.concatenate([np.asarray(W, np.float32), As, Ad], 1)

    w1c = wcat(W1, a1_src, a1_dst)
    w2c = wcat(W2, a2_src, a2_dst)
    xt = np.zeros((FEAT, cfg.npad), np.float32)
    xt[:, :n] = np.asarray(x, np.float32).T

    in_maps = []
    for k in range(ncores):
        s, e = core_lo[k], core_hi[k]
        csrc, cdst = src[s:e], dst[s:e]
        cloc = cdst - k * per
        cblk = cloc // 128
        S1 = np.zeros(ne, np.int32)
        D1 = np.zeros(ne, np.int32)
        S2 = np.zeros(ne, np.int32)
        D2 = np.zeros(ne, np.int32)
        DL = np.full(ne, -1.0, np.float32)
        bs = np.zeros(nb + 1, np.int64)
        np.add.at(bs[1:], cblk, 1)
        offs = np.cumsum(bs)[:-1]
        pos = (cblk * cfg.tb * 128) + (np.arange(len(csrc)) - offs[cblk])
        S1[pos] = csrc
        D1[pos] = cdst
        S2[pos] = remap[csrc]
        D2[pos] = remap[cdst]
        DL[pos] = (cloc % 128).astype(np.float32)

        def tposed(a):
            return np.ascontiguousarray(a.reshape(cfg.nt, 128).T)

        pid = np.full(cfg.sh, -1.0, np.float32)
        pw = np.zeros(cfg.sh, np.float32)
        lo, hi = k * per, (k + 1) * per
        pid[:per] = batch[lo:hi].astype(np.float32)
        pw[:per] = wg[batch[lo:hi]]
        in_maps.append({
            "XT": xt, "W1cat": w1c, "W2cat": w2c,
            "B1M": b1m, "B2M": b2m,
            "WL": np.asarray(Wl, np.float32),
            "BLC": np.asarray(bl, np.float32).reshape(OUT, 1),
            "SRC1": tposed(S1), "DST1": tposed(D1),
            "SRC2": tposed(S2), "DST2": tposed(D2),
            "DLOC": tposed(DL.view(np.int32)),
            "PID": np.ascontiguousarray(pid.reshape(nb, 128).T),
            "PW": np.ascontiguousarray(pw.reshape(nb, 128).T),
        })
    return cfg, in_maps


def build(cfg: Cfg):
    nc = bacc.Bacc("TRN2", target_bir_lowering=False, debug=False,
                   num_devices=cfg.ncores)
    NB, TB, NT = cfg.nb, cfg.tb, cfg.nt
    NP1 = cfg.npad // 128
    NP2 = cfg.n2 // 128

    ein = lambda nm, sh, dt: nc.dram_tensor(nm, sh, dt, kind="ExternalInput").ap()
    XT = ein("XT", [FEAT, cfg.npad], F32)
    W1c = ein("W1cat", [FEAT, 136], F32)
    W2c = ein("W2cat", [FEAT, 136], F32)
    B1M = ein("B1M", [128, FEAT], F32)
    B2M = ein("B2M", [128, FEAT], F32)
    WL = ein("WL", [FEAT, OUT], F32)
    BLC = ein("BLC", [OUT, 1], F32)
    SRC1 = ein("SRC1", [128, NT], I32)
    DST1 = ein("DST1", [128, NT], I32)
    SRC2 = ein("SRC2", [128, NT], I32)
    DST2 = ein("DST2", [128, NT], I32)
    DLOC = ein("DLOC", [128, NT], I32)
    PID = ein("PID", [128, NB], F32)
    PW = ein("PW", [128, NB], F32)
    OUTT = nc.dram_tensor("OUT", [64, OUT], F32, kind="ExternalOutput").ap()

    H1p = nc.dram_tensor("H1p", [cfg.npad, ROWB], U8).ap()
    h1sh = nc.dram_tensor("h1sh", [cfg.sh, FEAT], BF16).ap()
    h1full = nc.dram_tensor("h1full", [cfg.n2, FEAT], BF16,
                            addr_space="Shared").ap()
    H2p = nc.dram_tensor("H2p", [cfg.n2, ROWB], U8).ap()
    prt = nc.dram_tensor("prt", [FEAT, 64], F32).ap()
    prf = nc.dram_tensor("prf", [FEAT, 64], F32, addr_space="Shared").ap()

    groups = [list(range(cfg.ncores))]

    with tile.TileContext(nc) as tc:
        import contextlib
        ctx = contextlib.ExitStack()
        with ctx:
            consts = ctx.enter_context(tc.tile_pool(name="consts", bufs=1))
            xtp = ctx.enter_context(tc.tile_pool(name="xtp", bufs=4))
            hp = ctx.enter_context(tc.tile_pool(name="hp", bufs=4))
            hps = ctx.enter_context(tc.tile_pool(name="hps", bufs=3, space="PSUM"))
            gp = ctx.enter_context(tc.tile_pool(name="gp", bufs=4))
            sp = ctx.enter_context(tc.tile_pool(name="sp", bufs=4))
            up = ctx.enter_context(tc.tile_pool(name="up", bufs=3, space="PSUM"))
            ep = ctx.enter_context(tc.tile_pool(name="ep", bufs=4))
            pp = ctx.enter_context(tc.tile_pool(name="pp", bufs=1, space="PSUM"))
            tp = ctx.enter_context(tc.tile_pool(name="tp", bufs=1))

            w1sb = consts.tile([FEAT, 136], F32, tag="w1")
            nc.sync.dma_start(out=w1sb[:], in_=W1c)
            w1sb_b = consts.tile([FEAT, 136], BF16, tag="w1b")
            nc.vector.tensor_copy(out=w1sb_b[:], in_=w1sb[:])
            w2sb_f = consts.tile([FEAT, 136], F32, tag="w2f")
            nc.sync.dma_start(out=w2sb_f[:], in_=W2c)
            w2sb = consts.tile([FEAT, 136], BF16, tag="w2b")
            nc.vector.tensor_copy(out=w2sb[:], in_=w2sb_f[:])
            b1sb = consts.tile([128, FEAT], F32, tag="b1")
            nc.sync.dma_start(out=b1sb[:], in_=B1M)
            b2sb = consts.tile([128, FEAT], F32, tag="b2")
            nc.sync.dma_start(out=b2sb[:], in_=B2M)
            wlsb = consts.tile([FEAT, OUT], F32, tag="wl")
            nc.sync.dma_start(out=wlsb[:], in_=WL)
            blsb = consts.tile([OUT, 1], F32, tag="bl")
            nc.sync.dma_start(out=blsb[:], in_=BLC)
            ident = consts.tile([128, 128], F32, tag="ident")
            make_identity(nc, ident[:])
            iotaI = consts.tile([128, 128], I32, tag="iotai")
            nc.gpsimd.iota(iotaI[:], pattern=[[1, 128]], base=0,
                           channel_multiplier=0)
            iotaB = consts.tile([128, 128], BF16, tag="iotab")
            nc.vector.tensor_copy(out=iotaB[:], in_=iotaI[:])
            iota64 = consts.tile([128, 64], F32, tag="iota64")
            nc.vector.tensor_copy(out=iota64[:], in_=iotaI[:, :64])
            # dense [0..127] repeated TB times — dense src for the S build
            iotaR = consts.tile([128, TB * 128], BF16, tag="iotar")
            for t in range(TB):
                nc.vector.tensor_copy(out=iotaR[:, t * 128:(t + 1) * 128],
                                      in_=iotaB[:])
            pidsb = consts.tile([128, NB], F32, tag="pid")
            nc.sync.dma_start(out=pidsb[:], in_=PID)
            pwsb = consts.tile([128, NB], F32, tag="pw")
            nc.sync.dma_start(out=pwsb[:], in_=PW)

            idxp = ctx.enter_context(tc.tile_pool(name="idxp", bufs=1))
            src1 = idxp.tile([128, NT], I32, tag="src1")
            nc.sync.dma_start(out=src1[:], in_=SRC1)
            dst1 = idxp.tile([128, NT], I32, tag="dst1")
            nc.sync.dma_start(out=dst1[:], in_=DST1)
            src2 = idxp.tile([128, NT], I32, tag="src2")
            nc.sync.dma_start(out=src2[:], in_=SRC2)
            dst2 = idxp.tile([128, NT], I32, tag="dst2")
            nc.sync.dma_start(out=dst2[:], in_=DST2)
            dlocI = idxp.tile([128, NT], I32, tag="dloc")
            nc.sync.dma_start(out=dlocI[:], in_=DLOC)
            dlocF = dlocI[:].bitcast(F32)
            dlocB = idxp.tile([128, NT], BF16, tag="dlocb")
            nc.vector.tensor_copy(out=dlocB[:], in_=dlocF)

            def h_phase(nblocks, lhs_group, wsb, Hp):
                grp = cfg.xt_grp
                for m0 in range(0, nblocks, grp):
                    gi = m0 // grp
                    mcnt = min(grp, nblocks - m0)
                    lhs_of = lhs_group(m0, mcnt)
                    sb = hp.tile([128, grp * ROWB], U8, tag="hrow")
                    sbB = sb[:].bitcast(BF16)   # 144 bf16 per row
                    sbF = sb[:].bitcast(F32)    # 72 f32 per row
                    for j in range(mcnt):
                        ps = hps.tile([128, 136], F32, tag="hpsum")
                        nc.tensor.matmul(ps[:], lhsT=lhs_of(j), rhs=wsb[:],
                                         start=True, stop=True)
                        if j % 2 == 0:
                            nc.scalar.copy(out=sbB[:, j * 144:j * 144 + FEAT],
                                           in_=ps[:, :FEAT])
                        else:
                            nc.vector.tensor_copy(
                                out=sbB[:, j * 144:j * 144 + FEAT],
                                in_=ps[:, :FEAT])
                        nc.vector.tensor_copy(
                            out=sbF[:, j * 72 + 64:j * 72 + 72],
                            in_=ps[:, FEAT:FEAT + 2 * HEADS])
                    ho = Hp[m0 * 128:(m0 + mcnt) * 128, :].rearrange(
                        "(b p) c -> p b c", p=128)
                    weng = nc.scalar if gi % 2 == 0 else nc.sync
                    weng.dma_start(
                        out=ho, in_=sb[:, :mcnt * ROWB].rearrange(
                            "p (b c) -> p b c", c=ROWB))

            def p1_lhs(m0, mcnt):
                t = xtp.tile([128, cfg.xt_grp * 128], BF16, tag="xt")
                nc.gpsimd.dma_start(out=t[:, :mcnt * 128],
                                    in_=XT[:, m0 * 128:(m0 + mcnt) * 128])
                return lambda j: t[:, j * 128:(j + 1) * 128]

            h_phase(NP1, p1_lhs, w1sb_b, H1p)

            def edge_phase(srcT, dstT, Hp, layer):
                hgrp = cfg.h_grp
                h1grp = None
                g1 = g2 = None
                PAIR = 1  # blocks per gather instruction
                for b in range(NB):
                    if b % PAIR == 0:
                        bw = min(PAIR, NB - b)  # blocks in this gather
                        t0g = b * TB
                        g1 = gp.tile([128, PAIR * TB * ROWB], U8, tag="g1")
                        nc.gpsimd.indirect_dma_start(
                            out=g1[:, :bw * TB * ROWB], out_offset=None,
                            in_=Hp,
                            in_offset=bass.IndirectOffsetOnAxis(
                                ap=srcT[:, t0g:t0g + bw * TB], axis=0))
                        g2 = gp.tile([128, PAIR * TB * 4 * HEADS], U8, tag="g2")
                        nc.gpsimd.indirect_dma_start(
                            out=g2[:, :bw * TB * 4 * HEADS], out_offset=None,
                            in_=Hp,
                            in_offset=bass.IndirectOffsetOnAxis(
                                ap=dstT[:, t0g:t0g + bw * TB], axis=0),
                            element_offset=256 + 4 * HEADS)
                    sub = b % PAIR
                    t0 = b * TB
                    g1B = g1[:].bitcast(BF16)[
                        :, sub * TB * 144:(sub + 1) * TB * 144]
                    g1F = g1[:].bitcast(F32)[
                        :, sub * TB * 72:(sub + 1) * TB * 72]
                    g2s = g2[:].bitcast(F32)[
                        :, sub * TB * HEADS:(sub + 1) * TB * HEADS]
                    lg = sp.tile([128, TB * HEADS], F32, tag="lg")
                    als = g1F.rearrange("p (t c) -> p t c", c=72)[:, :, 64:68]
                    nc.vector.tensor_tensor(
                        out=lg[:].rearrange("p (t c) -> p t c", c=HEADS),
                        in0=als,
                        in1=g2s.rearrange("p (t c) -> p t c", c=HEADS),
                        op=mybir.AluOpType.add)
                    lg2 = sp.tile([128, TB * HEADS], F32, tag="lg2")
                    nc.vector.tensor_scalar_mul(lg2[:], lg[:], NEG_SLOPE)
                    nc.vector.tensor_tensor(out=lg[:], in0=lg[:], in1=lg2[:],
                                            op=mybir.AluOpType.max)
                    pv = g1B.rearrange("p (t c) -> p t c", c=144)[
                        :, :, FEAT:FEAT + HEADS]
                    nc.scalar.activation(
                        out=pv,
                        in_=lg[:].rearrange("p (t c) -> p t c", c=HEADS),
                        func=mybir.ActivationFunctionType.Exp)
                    # dense broadcast expansions on ScalarE so the two big
                    # VectorE tensor_tensor ops run in the 2x perf mode
                    dlocX = sp.tile([128, TB * 128], BF16, tag="dlocX")
                    nc.vector.tensor_copy(
                        out=dlocX[:].rearrange("p (t c) -> p t c", c=128),
                        in_=dlocB[:, t0:t0 + TB].unsqueeze(2).to_broadcast(
                            [128, TB, 128]))
                    pX = sp.tile([128, TB * 128], BF16, tag="pX")
                    nc.scalar.copy(
                        out=pX[:].rearrange("p (t h c) -> p t h c",
                                            h=HEADS, c=HID),
                        in_=g1B.rearrange("p (t c) -> p t c", c=144)[
                            :, :, FEAT:FEAT + HEADS].unsqueeze(3).to_broadcast(
                            [128, TB, HEADS, HID]))
                    S = sp.tile([128, TB * 128], BF16, tag="S")
                    nc.vector.tensor_tensor(
                        out=S[:], in0=iotaR[:], in1=dlocX[:],
                        op=mybir.AluOpType.is_equal)
                    hv = g1B.rearrange("p (t c) -> p t c", c=144)[:, :, :FEAT]
                    nc.vector.tensor_tensor(
                        out=hv, in0=hv,
                        in1=pX[:].rearrange("p (t c) -> p t c", c=128),
                        op=mybir.AluOpType.mult)
                    ups = up.tile([128, FEAT + HEADS], F32, tag="u")
                    for t in range(TB):
                        nc.tensor.matmul(
                            ups[:],
                            lhsT=S[:, t * 128:(t + 1) * 128],
                            rhs=g1B[:, t * 144:t * 144 + FEAT + HEADS],
                            start=(t == 0), stop=(t == TB - 1))
                    z = ep.tile([128, HEADS], F32, tag="z")
                    nc.vector.tensor_scalar_add(z[:], ups[:, FEAT:FEAT + HEADS],
                                                EPS)
                    rz = ep.tile([128, HEADS], F32, tag="rz")
                    nc.vector.reciprocal(rz[:], z[:])
                    o1 = ep.tile([128, FEAT], F32, tag="o1")
                    nc.vector.tensor_tensor(
                        out=o1[:].rearrange("p (h c) -> p h c", c=HID),
                        in0=ups[:, :FEAT].rearrange("p (h c) -> p h c", c=HID),
                        in1=rz[:].unsqueeze(2).to_broadcast([128, HEADS, HID]),
                        op=mybir.AluOpType.mult)
                    if layer == 1:
                        nc.vector.tensor_tensor(out=o1[:], in0=o1[:],
                                                in1=b1sb[:],
                                                op=mybir.AluOpType.add)
                        mn = ep.tile([128, FEAT], F32, tag="mn")
                        nc.vector.tensor_scalar_min(mn[:], o1[:], 0.0)
                        ex = ep.tile([128, FEAT], F32, tag="ex")
                        nc.scalar.activation(
                            out=ex[:], in_=mn[:],
                            func=mybir.ActivationFunctionType.Exp)
                        nc.vector.tensor_scalar(
                            o1[:], o1[:], 0.0, -1.0,
                            op0=mybir.AluOpType.max, op1=mybir.AluOpType.add)
                        if h1grp is None or b % hgrp == 0:
                            h1grp = ep.tile([128, hgrp * FEAT], BF16, tag="h1g")
                        nc.vector.tensor_tensor(
                            out=h1grp[:, (b % hgrp) * FEAT:
                                      (b % hgrp + 1) * FEAT],
                            in0=o1[:], in1=ex[:], op=mybir.AluOpType.add)
                        if b % hgrp == hgrp - 1 or b == NB - 1:
                            blo = (b // hgrp) * hgrp
                            bcnt = b - blo + 1
                            ho = h1sh[blo * 128:(b + 1) * 128, :].rearrange(
                                "(q p) c -> p q c", p=128)
                            nc.scalar.dma_start(
                                out=ho,
                                in_=h1grp[:, :bcnt * FEAT].rearrange(
                                    "p (q c) -> p q c", c=FEAT))
                    else:
                        h2 = ep.tile([128, FEAT], F32, tag="h2")
                        nc.vector.tensor_tensor(out=h2[:], in0=o1[:],
                                                in1=b2sb[:],
                                                op=mybir.AluOpType.add)
                        spg = ep.tile([128, 64], F32, tag="spg")
                        nc.vector.tensor_tensor(
                            out=spg[:],
                            in0=pidsb[:, b:b + 1].to_broadcast([128, 64]),
                            in1=iota64[:], op=mybir.AluOpType.is_equal)
                        nc.vector.tensor_scalar(
                            spg[:], spg[:], pwsb[:, b:b + 1], None,
                            op0=mybir.AluOpType.mult)
                        nc.tensor.matmul(
                            ppsum[:], lhsT=h2[:], rhs=spg[:],
                            start=(b == 0), stop=(b == NB - 1),
                            skip_group_check=True)

            edge_phase(src1, dst1, H1p, layer=1)

            nc.gpsimd.collective_compute(
                "AllGather", mybir.AluOpType.bypass, replica_groups=groups,
                ins=[h1sh.opt()], outs=[h1full.opt()])

            def p3_lhs(m0, mcnt):
                t = xtp.tile([128, cfg.xt_grp * 128], BF16, tag="h1t")
                nc.sync.dma_start(
                    out=t[:, :mcnt * 128],
                    in_=h1full[m0 * 128:(m0 + mcnt) * 128, :],
                    transpose=True)
                return lambda j: t[:, j * 128:(j + 1) * 128]

            h_phase(NP2, p3_lhs, w2sb, H2p)

            ppsum = pp.tile([128, 64], F32, tag="pool")
            edge_phase(src2, dst2, H2p, layer=2)

            psb = tp.tile([128, 64], F32, tag="psb")
            nc.vector.tensor_copy(out=psb[:], in_=ppsum[:])
            nc.sync.dma_start(out=prt, in_=psb[:])
            nc.gpsimd.collective_compute(
                "AllReduce", mybir.AluOpType.add, replica_groups=groups,
                ins=[prt.opt()], outs=[prf.opt()])
            pall = tp.tile([128, 64], F32, tag="pall")
            nc.sync.dma_start(out=pall[:], in_=prf)
            lps = up.tile([OUT, 64], F32, tag="u")
            nc.tensor.matmul(lps[:], lhsT=wlsb[:], rhs=pall[:],
                             start=True, stop=True)
            lsb = tp.tile([OUT, 64], F32, tag="lsb")
            nc.vector.tensor_scalar(lsb[:], lps[:], blsb[:, :1], None,
                                    op0=mybir.AluOpType.add)
            tps = up.tile([64, OUT], F32, tag="u")
            nc.tensor.transpose(out=tps[:], in_=lsb[:],
                                identity=ident[:OUT, :OUT])
            sm = tp.tile([64, OUT], F32, tag="sm")
            nc.vector.tensor_copy(out=sm[:], in_=tps[:])
            mx = tp.tile([64, 1], F32, tag="mx")
            nc.vector.reduce_max(mx[:], sm[:], axis=mybir.AxisListType.X)
            nc.vector.tensor_scalar(sm[:], sm[:], mx[:, :1], None,
                                    op0=mybir.AluOpType.subtract)
            nc.scalar.activation(out=sm[:], in_=sm[:],
                                 func=mybir.ActivationFunctionType.Exp)
            ssum = tp.tile([64, 1], F32, tag="ssum")
            nc.vector.reduce_sum(ssum[:], sm[:], axis=mybir.AxisListType.X)
            rs = tp.tile([64, 1], F32, tag="rs")
            nc.vector.reciprocal(rs[:], ssum[:])
            nc.vector.tensor_scalar(sm[:], sm[:], rs[:, :1], None,
                                    op0=mybir.AluOpType.mult)
            nc.sync.dma_start(out=OUTT, in_=sm[:])

    nc.compile()
    return nc


_CACHE = {}


def kernel(**inputs) -> np.ndarray:
    ncores = 8
    cfg, in_maps = host_prep(ncores=ncores, **inputs)
    key = dataclasses.astuple(cfg)
    if key not in _CACHE:
        _CACHE[key] = build(cfg)
    nc = _CACHE[key]
    res = bass_utils.run_bass_kernel_spmd(nc, in_maps,
                                          core_ids=list(range(ncores)))
    out = res.results[0]["OUT"][:64]
    return np.asarray(out, np.float32)


if __name__ == "__main__":
    # quick self-run with random data matching the spec
    rng = np.random.default_rng(0)
    ins = {
        "x": rng.standard_normal((50000, 128), np.float32),
        "W1": (rng.standard_normal((128, 128)) * 0.05).astype(np.float32),
        "a1_src": (rng.standard_normal((4, 32)) * 0.05).astype(np.float32),
        "a1_dst": (rng.standard_normal((4, 32)) * 0.05).astype(np.float32),
        "b1": np.zeros(128, np.float32),
        "W2": (rng.standard_normal((128, 128)) * 0.05).astype(np.float32),
        "a2_src": (rng.standard_normal((4, 32)) * 0.05).astype(np.float32),
        "a2_dst": (rng.standard_normal((4, 32)) * 0.05).astype(np.float32),
        "b2": np.zeros(128, np.float32),
        "Wl": (rng.standard_normal((128, 10)) * 0.05).astype(np.float32),
        "bl": np.zeros(10, np.float32),
        "edge_index": rng.integers(0, 50000, (2, 800000)).astype(np.int32),
        "batch": np.sort(rng.integers(0, 64, 50000)).astype(np.int32),
    }
    out = kernel(**ins)
    print(out.shape, out.dtype, out[:2])


# revision 19
# speedup vs baseline: 1.6708x; 1.0567x over previous
"""2-layer GAT + global mean pool + linear + softmax on 8 Trainium2 cores.

Self-contained Bass/Tile kernel. Sharding: dst-nodes uniformly across the 8
cores; edges sorted by dst; every 128-dst block padded to a uniform tile
count so one SPMD instruction stream serves all cores. h1 is AllGather'ed
between the layers; pooled partials are AllReduce'd at the end.
"""

import dataclasses
import sys

import numpy as np

for _p in ("/opt/trn_rl_repo", "/opt/trn_rl_repo/concourse"):
    if _p not in sys.path:
        sys.path.insert(0, _p)

import concourse.bass as bass
import concourse.bacc as bacc
import concourse.mybir as mybir
import concourse.tile as tile
from concourse import bass_utils
from concourse.masks import make_identity

F32 = mybir.dt.float32
BF16 = mybir.dt.bfloat16
I32 = mybir.dt.int32
U8 = mybir.dt.uint8

HEADS, HID, FEAT, OUT = 4, 32, 128, 10
ROWB = 288  # H' row bytes: 128 bf16 | 4 f32 al_src | 4 f32 al_dst
NEG_SLOPE = 0.2
EPS = 1e-16


@dataclasses.dataclass
class Cfg:
    ncores: int
    n: int
    g: int
    nb: int          # dst blocks per core
    tb: int          # tiles (128 edges) per block — uniform across cores
    xt_grp: int = 8
    h_grp: int = 4

    @property
    def npad(self):
        return ((self.n + 127) // 128) * 128

    @property
    def sh(self):
        return self.nb * 128

    @property
    def n2(self):
        return self.ncores * self.sh

    @property
    def nt(self):
        return self.nb * self.tb

    @property
    def ne(self):
        return self.nt * 128


def host_prep(x, W1, a1_src, a1_dst, b1, W2, a2_src, a2_dst, b2, Wl, bl,
              edge_index, batch, ncores):
    n = x.shape[0]
    g = 64
    e0 = np.asarray(edge_index[0], np.int64)
    e1 = np.asarray(edge_index[1], np.int64)
    batch = np.asarray(batch, np.int64)
    loops = np.arange(n, dtype=np.int64)
    src = np.concatenate([e0, loops])
    dst = np.concatenate([e1, loops])
    order = np.argsort(dst, kind="stable")
    src, dst = src[order], dst[order]

    assert n % ncores == 0, (n, ncores)
    per = n // ncores
    nb = (per + 127) // 128

    counts = np.zeros((ncores, nb), np.int64)
    core_of = dst // per
    loc = dst - core_of * per
    blk = loc // 128
    np.add.at(counts, (core_of, blk), 1)
    tb = int(np.ceil(counts.max() / 128))
    cfg = Cfg(ncores=ncores, n=n, g=g, nb=nb, tb=tb)

    cnt_g = np.bincount(batch, minlength=g).astype(np.float64)
    wg = (1.0 / np.maximum(cnt_g, 1.0)).astype(np.float32)

    node_core = np.arange(n) // per
    node_loc = np.arange(n) - node_core * per
    remap = node_core * cfg.sh + node_loc

    ne = cfg.ne
    core_lo = np.searchsorted(dst, np.arange(ncores) * per)
    core_hi = np.searchsorted(dst, (np.arange(ncores) + 1) * per)

    b1m = np.broadcast_to(np.asarray(b1, np.float32), (128, FEAT)).copy()
    b2m = np.broadcast_to(np.asarray(b2, np.float32), (128, FEAT)).copy()

    def wcat(W, a_s, a_d):
        As = np.zeros((FEAT, HEADS), np.float32)
        Ad = np.zeros((FEAT, HEADS), np.float32)
        for h in range(HEADS):
            As[h * HID:(h + 1) * HID, h] = a_s[h]
            Ad[h * HID:(h + 1) * HID, h] = a_d[h]
        return np.concatenate([np.asarray(W, np.float32), As, Ad], 1)

    w1c = wcat(W1, a1_src, a1_dst)
    w2c = wcat(W2, a2_src, a2_dst)
    xt = np.zeros((FEAT, cfg.npad), np.float32)
    xt[:, :n] = np.asarray(x, np.float32).T

    in_maps = []
    for k in range(ncores):
        s, e = core_lo[k], core_hi[k]
        csrc, cdst = src[s:e], dst[s:e]
        cloc = cdst - k * per
        cblk = cloc // 128
        S1 = np.zeros(ne, np.int32)
        D1 = np.zeros(ne, np.int32)
        S2 = np.zeros(ne, np.int32)
        D2 = np.zeros(ne, np.int32)
        DL = np.full(ne, -1.0, np.float32)
        bs = np.zeros(nb + 1, np.int64)
        np.add.at(bs[1:], cblk, 1)
        offs = np.cumsum(bs)[:-1]
        pos = (cblk * cfg.tb * 128) + (np.arange(len(csrc)) - offs[cblk])
        S1[pos] = csrc
        D1[pos] = cdst
        S2[pos] = remap[csrc]
        D2[pos] = remap[cdst]
        DL[pos] = (cloc % 128).astype(np.float32)

        def tposed(a):
            return np.ascontiguousarray(a.reshape(cfg.nt, 128).T)

        # one-hot S: SH[p, t*128+d] = (DL[t*128+p] == d), fp8 (exact 0/1)
        import ml_dtypes
        dlt = tposed(DL)  # [128, nt]
        sh_ = (dlt[:, :, None] == np.arange(128, dtype=np.float32)[None, None, :])
        SH = sh_.reshape(128, cfg.nt * 128).astype(ml_dtypes.float8_e4m3)

        pid = np.full(cfg.sh, -1.0, np.float32)
        pw = np.zeros(cfg.sh, np.float32)
        lo, hi = k * per, (k + 1) * per
        pid[:per] = batch[lo:hi].astype(np.float32)
        pw[:per] = wg[batch[lo:hi]]
        in_maps.append({
            "XT": xt, "W1cat": w1c, "W2cat": w2c,
            "B1M": b1m, "B2M": b2m,
            "WL": np.asarray(Wl, np.float32),
            "BLC": np.asarray(bl, np.float32).reshape(OUT, 1),
            "SRC1": tposed(S1), "DST1": tposed(D1),
            "SRC2": tposed(S2), "DST2": tposed(D2),
            "SH": SH,
            "PID": np.ascontiguousarray(pid.reshape(nb, 128).T),
            "PW": np.ascontiguousarray(pw.reshape(nb, 128).T),
        })
    return cfg, in_maps


def build(cfg: Cfg):
    nc = bacc.Bacc("TRN2", target_bir_lowering=False, debug=False,
                   num_devices=cfg.ncores)
    NB, TB, NT = cfg.nb, cfg.tb, cfg.nt
    NP1 = cfg.npad // 128
    NP2 = cfg.n2 // 128

    ein = lambda nm, sh, dt: nc.dram_tensor(nm, sh, dt, kind="ExternalInput").ap()
    XT = ein("XT", [FEAT, cfg.npad], F32)
    W1c = ein("W1cat", [FEAT, 136], F32)
    W2c = ein("W2cat", [FEAT, 136], F32)
    B1M = ein("B1M", [128, FEAT], F32)
    B2M = ein("B2M", [128, FEAT], F32)
    WL = ein("WL", [FEAT, OUT], F32)
    BLC = ein("BLC", [OUT, 1], F32)
    SRC1 = ein("SRC1", [128, NT], I32)
    DST1 = ein("DST1", [128, NT], I32)
    SRC2 = ein("SRC2", [128, NT], I32)
    DST2 = ein("DST2", [128, NT], I32)
    F8 = mybir.dt.float8e4
    SH = ein("SH", [128, NT * 128], F8)
    PID = ein("PID", [128, NB], F32)
    PW = ein("PW", [128, NB], F32)
    OUTT = nc.dram_tensor("OUT", [64, OUT], F32, kind="ExternalOutput").ap()

    H1p = nc.dram_tensor("H1p", [cfg.npad, ROWB], U8).ap()
    h1sh = nc.dram_tensor("h1sh", [cfg.sh, FEAT], BF16).ap()
    h1full = nc.dram_tensor("h1full", [cfg.n2, FEAT], BF16,
                            addr_space="Shared").ap()
    H2p = nc.dram_tensor("H2p", [cfg.n2, ROWB], U8).ap()
    prt = nc.dram_tensor("prt", [FEAT, 64], F32).ap()
    prf = nc.dram_tensor("prf", [FEAT, 64], F32, addr_space="Shared").ap()

    groups = [list(range(cfg.ncores))]

    with tile.TileContext(nc) as tc:
        import contextlib
        ctx = contextlib.ExitStack()
        with ctx:
            consts = ctx.enter_context(tc.tile_pool(name="consts", bufs=1))
            xtp = ctx.enter_context(tc.tile_pool(name="xtp", bufs=4))
            hp = ctx.enter_context(tc.tile_pool(name="hp", bufs=4))
            hps = ctx.enter_context(tc.tile_pool(name="hps", bufs=3, space="PSUM"))
            gp = ctx.enter_context(tc.tile_pool(name="gp", bufs=4))
            sp = ctx.enter_context(tc.tile_pool(name="sp", bufs=4))
            up = ctx.enter_context(tc.tile_pool(name="up", bufs=3, space="PSUM"))
            ep = ctx.enter_context(tc.tile_pool(name="ep", bufs=4))
            pp = ctx.enter_context(tc.tile_pool(name="pp", bufs=1, space="PSUM"))
            tp = ctx.enter_context(tc.tile_pool(name="tp", bufs=1))

            w1sb = consts.tile([FEAT, 136], F32, tag="w1")
            nc.sync.dma_start(out=w1sb[:], in_=W1c)
            w1sb_b = consts.tile([FEAT, 136], BF16, tag="w1b")
            nc.vector.tensor_copy(out=w1sb_b[:], in_=w1sb[:])
            w2sb_f = consts.tile([FEAT, 136], F32, tag="w2f")
            nc.sync.dma_start(out=w2sb_f[:], in_=W2c)
            w2sb = consts.tile([FEAT, 136], BF16, tag="w2b")
            nc.vector.tensor_copy(out=w2sb[:], in_=w2sb_f[:])
            b1sb = consts.tile([128, FEAT], F32, tag="b1")
            nc.sync.dma_start(out=b1sb[:], in_=B1M)
            b2sb = consts.tile([128, FEAT], F32, tag="b2")
            nc.sync.dma_start(out=b2sb[:], in_=B2M)
            wlsb = consts.tile([FEAT, OUT], F32, tag="wl")
            nc.sync.dma_start(out=wlsb[:], in_=WL)
            blsb = consts.tile([OUT, 1], F32, tag="bl")
            nc.sync.dma_start(out=blsb[:], in_=BLC)
            ident = consts.tile([128, 128], F32, tag="ident")
            make_identity(nc, ident[:])
            iotaI = consts.tile([128, 128], I32, tag="iotai")
            nc.gpsimd.iota(iotaI[:], pattern=[[1, 128]], base=0,
                           channel_multiplier=0)
            iotaB = consts.tile([128, 128], BF16, tag="iotab")
            nc.vector.tensor_copy(out=iotaB[:], in_=iotaI[:])
            iota64 = consts.tile([128, 64], F32, tag="iota64")
            nc.vector.tensor_copy(out=iota64[:], in_=iotaI[:, :64])
            pidsb = consts.tile([128, NB], F32, tag="pid")
            nc.sync.dma_start(out=pidsb[:], in_=PID)
            pwsb = consts.tile([128, NB], F32, tag="pw")
            nc.sync.dma_start(out=pwsb[:], in_=PW)

            idxp = ctx.enter_context(tc.tile_pool(name="idxp", bufs=1))
            src1 = idxp.tile([128, NT], I32, tag="src1")
            nc.sync.dma_start(out=src1[:], in_=SRC1)
            dst1 = idxp.tile([128, NT], I32, tag="dst1")
            nc.sync.dma_start(out=dst1[:], in_=DST1)
            src2 = idxp.tile([128, NT], I32, tag="src2")
            nc.sync.dma_start(out=src2[:], in_=SRC2)
            dst2 = idxp.tile([128, NT], I32, tag="dst2")
            nc.sync.dma_start(out=dst2[:], in_=DST2)

            def h_phase(nblocks, lhs_group, wsb, Hp):
                grp = cfg.xt_grp
                for m0 in range(0, nblocks, grp):
                    gi = m0 // grp
                    mcnt = min(grp, nblocks - m0)
                    lhs_of = lhs_group(m0, mcnt)
                    sb = hp.tile([128, grp * ROWB], U8, tag="hrow")
                    sbB = sb[:].bitcast(BF16)   # 144 bf16 per row
                    sbF = sb[:].bitcast(F32)    # 72 f32 per row
                    for j in range(mcnt):
                        ps = hps.tile([128, 136], F32, tag="hpsum")
                        nc.tensor.matmul(ps[:], lhsT=lhs_of(j), rhs=wsb[:],
                                         start=True, stop=True)
                        if j % 2 == 0:
                            nc.scalar.copy(out=sbB[:, j * 144:j * 144 + FEAT],
                                           in_=ps[:, :FEAT])
                        else:
                            nc.vector.tensor_copy(
                                out=sbB[:, j * 144:j * 144 + FEAT],
                                in_=ps[:, :FEAT])
                        nc.vector.tensor_copy(
                            out=sbF[:, j * 72 + 64:j * 72 + 72],
                            in_=ps[:, FEAT:FEAT + 2 * HEADS])
                    ho = Hp[m0 * 128:(m0 + mcnt) * 128, :].rearrange(
                        "(b p) c -> p b c", p=128)
                    weng = nc.scalar if gi % 2 == 0 else nc.sync
                    weng.dma_start(
                        out=ho, in_=sb[:, :mcnt * ROWB].rearrange(
                            "p (b c) -> p b c", c=ROWB))

            def p1_lhs(m0, mcnt):
                t = xtp.tile([128, cfg.xt_grp * 128], BF16, tag="xt")
                nc.gpsimd.dma_start(out=t[:, :mcnt * 128],
                                    in_=XT[:, m0 * 128:(m0 + mcnt) * 128])
                return lambda j: t[:, j * 128:(j + 1) * 128]

            h_phase(NP1, p1_lhs, w1sb_b, H1p)

            def edge_phase(srcT, dstT, Hp, layer):
                hgrp = cfg.h_grp
                h1grp = None
                g1 = g2 = None
                PAIR = 1  # blocks per gather instruction
                for b in range(NB):
                    if b % PAIR == 0:
                        bw = min(PAIR, NB - b)  # blocks in this gather
                        t0g = b * TB
                        g1 = gp.tile([128, PAIR * TB * ROWB], U8, tag="g1")
                        nc.gpsimd.indirect_dma_start(
                            out=g1[:, :bw * TB * ROWB], out_offset=None,
                            in_=Hp,
                            in_offset=bass.IndirectOffsetOnAxis(
                                ap=srcT[:, t0g:t0g + bw * TB], axis=0))
                        g2 = gp.tile([128, PAIR * TB * 4 * HEADS], U8, tag="g2")
                        nc.gpsimd.indirect_dma_start(
                            out=g2[:, :bw * TB * 4 * HEADS], out_offset=None,
                            in_=Hp,
                            in_offset=bass.IndirectOffsetOnAxis(
                                ap=dstT[:, t0g:t0g + bw * TB], axis=0),
                            element_offset=256 + 4 * HEADS)
                    sub = b % PAIR
                    t0 = b * TB
                    g1B = g1[:].bitcast(BF16)[
                        :, sub * TB * 144:(sub + 1) * TB * 144]
                    g1F = g1[:].bitcast(F32)[
                        :, sub * TB * 72:(sub + 1) * TB * 72]
                    g2s = g2[:].bitcast(F32)[
                        :, sub * TB * HEADS:(sub + 1) * TB * HEADS]
                    lg = sp.tile([128, TB * HEADS], F32, tag="lg")
                    als = g1F.rearrange("p (t c) -> p t c", c=72)[:, :, 64:68]
                    nc.vector.tensor_tensor(
                        out=lg[:].rearrange("p (t c) -> p t c", c=HEADS),
                        in0=als,
                        in1=g2s.rearrange("p (t c) -> p t c", c=HEADS),
                        op=mybir.AluOpType.add)
                    lg2 = sp.tile([128, TB * HEADS], F32, tag="lg2")
                    nc.vector.tensor_scalar_mul(lg2[:], lg[:], NEG_SLOPE)
                    nc.vector.tensor_tensor(out=lg[:], in0=lg[:], in1=lg2[:],
                                            op=mybir.AluOpType.max)
                    pv = g1B.rearrange("p (t c) -> p t c", c=144)[
                        :, :, FEAT:FEAT + HEADS]
                    nc.scalar.activation(
                        out=pv,
                        in_=lg[:].rearrange("p (t c) -> p t c", c=HEADS),
                        func=mybir.ActivationFunctionType.Exp)
                    # p expanded densely on ScalarE so the big VectorE
                    # tensor_tensor runs in the 2x perf mode
                    pX = sp.tile([128, TB * 128], BF16, tag="pX")
                    nc.scalar.copy(
                        out=pX[:].rearrange("p (t h c) -> p t h c",
                                            h=HEADS, c=HID),
                        in_=g1B.rearrange("p (t c) -> p t c", c=144)[
                            :, :, FEAT:FEAT + HEADS].unsqueeze(3).to_broadcast(
                            [128, TB, HEADS, HID]))
                    S = sp.tile([128, TB * 128], F8, tag="S")
                    nc.sync.dma_start(
                        out=S[:], in_=SH[:, t0 * 128:(t0 + TB) * 128])
                    hv = g1B.rearrange("p (t c) -> p t c", c=144)[:, :, :FEAT]
                    nc.vector.tensor_tensor(
                        out=hv, in0=hv,
                        in1=pX[:].rearrange("p (t c) -> p t c", c=128),
                        op=mybir.AluOpType.mult)
                    ups = up.tile([128, FEAT + HEADS], F32, tag="u")
                    for t in range(TB):
                        nc.tensor.matmul(
                            ups[:],
                            lhsT=S[:, t * 128:(t + 1) * 128],
                            rhs=g1B[:, t * 144:t * 144 + FEAT + HEADS],
                            start=(t == 0), stop=(t == TB - 1))
                    z = ep.tile([128, HEADS], F32, tag="z")
                    nc.vector.tensor_scalar_add(z[:], ups[:, FEAT:FEAT + HEADS],
                                                EPS)
                    rz = ep.tile([128, HEADS], F32, tag="rz")
                    nc.vector.reciprocal(rz[:], z[:])
                    o1 = ep.tile([128, FEAT], F32, tag="o1")
                    nc.vector.tensor_tensor(
                        out=o1[:].rearrange("p (h c) -> p h c", c=HID),
                        in0=ups[:, :FEAT].rearrange("p (h c) -> p h c", c=HID),
                        in1=rz[:].unsqueeze(2).to_broadcast([128, HEADS, HID]),
                        op=mybir.AluOpType.mult)
                    if layer == 1:
                        nc.vector.tensor_tensor(out=o1[:], in0=o1[:],
                                                in1=b1sb[:],
                                                op=mybir.AluOpType.add)
                        mn = ep.tile([128, FEAT], F32, tag="mn")
                        nc.vector.tensor_scalar_min(mn[:], o1[:], 0.0)
                        ex = ep.tile([128, FEAT], F32, tag="ex")
                        nc.scalar.activation(
                            out=ex[:], in_=mn[:],
                            func=mybir.ActivationFunctionType.Exp)
                        nc.vector.tensor_scalar(
                            o1[:], o1[:], 0.0, -1.0,
                            op0=mybir.AluOpType.max, op1=mybir.AluOpType.add)
                        if h1grp is None or b % hgrp == 0:
                            h1grp = ep.tile([128, hgrp * FEAT], BF16, tag="h1g")
                        nc.vector.tensor_tensor(
                            out=h1grp[:, (b % hgrp) * FEAT:
                                      (b % hgrp + 1) * FEAT],
                            in0=o1[:], in1=ex[:], op=mybir.AluOpType.add)
                        if b % hgrp == hgrp - 1 or b == NB - 1:
                            blo = (b // hgrp) * hgrp
                            bcnt = b - blo + 1
                            ho = h1sh[blo * 128:(b + 1) * 128, :].rearrange(
                                "(q p) c -> p q c", p=128)
                            nc.scalar.dma_start(
                                out=ho,
                                in_=h1grp[:, :bcnt * FEAT].rearrange(
                                    "p (q c) -> p q c", c=FEAT))
                    else:
                        h2 = ep.tile([128, FEAT], F32, tag="h2")
                        nc.vector.tensor_tensor(out=h2[:], in0=o1[:],
                                                in1=b2sb[:],
                                                op=mybir.AluOpType.add)
                        spg = ep.tile([128, 64], F32, tag="spg")
                        nc.vector.tensor_tensor(
                            out=spg[:],
                            in0=pidsb[:, b:b + 1].to_broadcast([128, 64]),
                            in1=iota64[:], op=mybir.AluOpType.is_equal)
                        nc.vector.tensor_scalar(
                            spg[:], spg[:], pwsb[:, b:b + 1], None,
                            op0=mybir.AluOpType.mult)
                        nc.tensor.matmul(
                            ppsum[:], lhsT=h2[:], rhs=spg[:],
                            start=(b == 0), stop=(b == NB - 1),
                            skip_group_check=True)

            edge_phase(src1, dst1, H1p, layer=1)

            nc.gpsimd.collective_compute(
                "AllGather", mybir.AluOpType.bypass, replica_groups=groups,
                ins=[h1sh.opt()], outs=[h1full.opt()])

            def p3_lhs(m0, mcnt):
                t = xtp.tile([128, cfg.xt_grp * 128], BF16, tag="h1t")
                nc.sync.dma_start(
                    out=t[:, :mcnt * 128],
                    in_=h1full[m0 * 128:(m0 + mcnt) * 128, :],
                    transpose=True)
                return lambda j: t[:, j * 128:(j + 1) * 128]

            h_phase(NP2, p3_lhs, w2sb, H2p)

            ppsum = pp.tile([128, 64], F32, tag="pool")
            edge_phase(src2, dst2, H2p, layer=2)

            psb = tp.tile([128, 64], F32, tag="psb")
            nc.vector.tensor_copy(out=psb[:], in_=ppsum[:])
            nc.sync.dma_start(out=prt, in_=psb[:])
            nc.gpsimd.collective_compute(
                "AllReduce", mybir.AluOpType.add, replica_groups=groups,
                ins=[prt.opt()], outs=[prf.opt()])
            pall = tp.tile([128, 64], F32, tag="pall")
            nc.sync.dma_start(out=pall[:], in_=prf)
            lps = up.tile([OUT, 64], F32, tag="u")
            nc.tensor.matmul(lps[:], lhsT=wlsb[:], rhs=pall[:],
                             start=True, stop=True)
            lsb = tp.tile([OUT, 64], F32, tag="lsb")
            nc.vector.tensor_scalar(lsb[:], lps[:], blsb[:, :1], None,
                                    op0=mybir.AluOpType.add)
            tps = up.tile([64, OUT], F32, tag="u")
            nc.tensor.transpose(out=tps[:], in_=lsb[:],
                                identity=ident[:OUT, :OUT])
            sm = tp.tile([64, OUT], F32, tag="sm")
            nc.vector.tensor_copy(out=sm[:], in_=tps[:])
            mx = tp.tile([64, 1], F32, tag="mx")
            nc.vector.reduce_max(mx[:], sm[:], axis=mybir.AxisListType.X)
            nc.vector.tensor_scalar(sm[:], sm[:], mx[:, :1], None,
                                    op0=mybir.AluOpType.subtract)
            nc.scalar.activation(out=sm[:], in_=sm[:],
                                 func=mybir.ActivationFunctionType.Exp)
            ssum = tp.tile([64, 1], F32, tag="ssum")
            nc.vector.reduce_sum(ssum[:], sm[:], axis=mybir.AxisListType.X)
            rs = tp.tile([64, 1], F32, tag="rs")
            nc.vector.reciprocal(rs[:], ssum[:])
            nc.vector.tensor_scalar(sm[:], sm[:], rs[:, :1], None,
                                    op0=mybir.AluOpType.mult)
            nc.sync.dma_start(out=OUTT, in_=sm[:])

    nc.compile()
    return nc


_CACHE = {}


def kernel(**inputs) -> np.ndarray:
    ncores = 8
    cfg, in_maps = host_prep(ncores=ncores, **inputs)
    key = dataclasses.astuple(cfg)
    if key not in _CACHE:
        _CACHE[key] = build(cfg)
    nc = _CACHE[key]
    res = bass_utils.run_bass_kernel_spmd(nc, in_maps,
                                          core_ids=list(range(ncores)))
    out = res.results[0]["OUT"][:64]
    return np.asarray(out, np.float32)


if __name__ == "__main__":
    # quick self-run with random data matching the spec
    rng = np.random.default_rng(0)
    ins = {
        "x": rng.standard_normal((50000, 128), np.float32),
        "W1": (rng.standard_normal((128, 128)) * 0.05).astype(np.float32),
        "a1_src": (rng.standard_normal((4, 32)) * 0.05).astype(np.float32),
        "a1_dst": (rng.standard_normal((4, 32)) * 0.05).astype(np.float32),
        "b1": np.zeros(128, np.float32),
        "W2": (rng.standard_normal((128, 128)) * 0.05).astype(np.float32),
        "a2_src": (rng.standard_normal((4, 32)) * 0.05).astype(np.float32),
        "a2_dst": (rng.standard_normal((4, 32)) * 0.05).astype(np.float32),
        "b2": np.zeros(128, np.float32),
        "Wl": (rng.standard_normal((128, 10)) * 0.05).astype(np.float32),
        "bl": np.zeros(10, np.float32),
        "edge_index": rng.integers(0, 50000, (2, 800000)).astype(np.int32),
        "batch": np.sort(rng.integers(0, 64, 50000)).astype(np.int32),
    }
    out = kernel(**ins)
    print(out.shape, out.dtype, out[:2])


# revision 21
# speedup vs baseline: 1.9475x; 1.1656x over previous
"""2-layer GAT + global mean pool + linear + softmax on 8 Trainium2 cores.

Self-contained Bass/Tile kernel. Sharding: dst-nodes uniformly across the 8
cores; edges sorted by dst; every 128-dst block padded to a uniform tile
count so one SPMD instruction stream serves all cores. h1 is AllGather'ed
between the layers; pooled partials are AllReduce'd at the end.
"""

import dataclasses
import sys

import numpy as np

for _p in ("/opt/trn_rl_repo", "/opt/trn_rl_repo/concourse"):
    if _p not in sys.path:
        sys.path.insert(0, _p)

import concourse.bass as bass
import concourse.bacc as bacc
import concourse.mybir as mybir
import concourse.tile as tile
from concourse import bass_utils
from concourse.masks import make_identity

F32 = mybir.dt.float32
BF16 = mybir.dt.bfloat16
I32 = mybir.dt.int32
U8 = mybir.dt.uint8

HEADS, HID, FEAT, OUT = 4, 32, 128, 10
ROWB = 288  # H' row bytes: 128 bf16 | 4 f32 al_src | 4 f32 al_dst
NEG_SLOPE = 0.2
EPS = 1e-16


@dataclasses.dataclass
class Cfg:
    ncores: int
    n: int
    g: int
    nb: int          # dst blocks per core
    tb: int          # tiles (128 edges) per block — uniform across cores
    xt_grp: int = 16
    h_grp: int = 4

    @property
    def npad(self):
        return ((self.n + 127) // 128) * 128

    @property
    def sh(self):
        return self.nb * 128

    @property
    def n2(self):
        return self.ncores * self.sh

    @property
    def nt(self):
        return self.nb * self.tb

    @property
    def ne(self):
        return self.nt * 128


def host_prep(x, W1, a1_src, a1_dst, b1, W2, a2_src, a2_dst, b2, Wl, bl,
              edge_index, batch, ncores):
    n = x.shape[0]
    g = 64
    e0 = np.asarray(edge_index[0], np.int64)
    e1 = np.asarray(edge_index[1], np.int64)
    batch = np.asarray(batch, np.int64)
    loops = np.arange(n, dtype=np.int64)
    src = np.concatenate([e0, loops])
    dst = np.concatenate([e1, loops])
    order = np.argsort(dst, kind="stable")
    src, dst = src[order], dst[order]

    assert n % ncores == 0, (n, ncores)
    per = n // ncores
    nb = (per + 127) // 128

    counts = np.zeros((ncores, nb), np.int64)
    core_of = dst // per
    loc = dst - core_of * per
    blk = loc // 128
    np.add.at(counts, (core_of, blk), 1)
    tb = int(np.ceil(counts.max() / 128))
    cfg = Cfg(ncores=ncores, n=n, g=g, nb=nb, tb=tb)

    cnt_g = np.bincount(batch, minlength=g).astype(np.float64)
    wg = (1.0 / np.maximum(cnt_g, 1.0)).astype(np.float32)

    node_core = np.arange(n) // per
    node_loc = np.arange(n) - node_core * per
    remap = node_core * cfg.sh + node_loc

    ne = cfg.ne
    core_lo = np.searchsorted(dst, np.arange(ncores) * per)
    core_hi = np.searchsorted(dst, (np.arange(ncores) + 1) * per)

    b1m = np.broadcast_to(np.asarray(b1, np.float32), (128, FEAT)).copy()
    b2m = np.broadcast_to(np.asarray(b2, np.float32), (128, FEAT)).copy()

    def wcat(W, a_s, a_d):
        As = np.zeros((FEAT, HEADS), np.float32)
        Ad = np.zeros((FEAT, HEADS), np.float32)
        for h in range(HEADS):
            As[h * HID:(h + 1) * HID, h] = a_s[h]
            Ad[h * HID:(h + 1) * HID, h] = a_d[h]
        return np.concatenate([np.asarray(W, np.float32), As, Ad], 1)

    w1c = wcat(W1, a1_src, a1_dst)
    w2c = wcat(W2, a2_src, a2_dst)
    xt = np.zeros((FEAT, cfg.npad), np.float32)
    xt[:, :n] = np.asarray(x, np.float32).T

    in_maps = []
    for k in range(ncores):
        s, e = core_lo[k], core_hi[k]
        csrc, cdst = src[s:e], dst[s:e]
        cloc = cdst - k * per
        cblk = cloc // 128
        S1 = np.zeros(ne, np.int32)
        D1 = np.zeros(ne, np.int32)
        S2 = np.zeros(ne, np.int32)
        D2 = np.zeros(ne, np.int32)
        DL = np.full(ne, -1.0, np.float32)
        bs = np.zeros(nb + 1, np.int64)
        np.add.at(bs[1:], cblk, 1)
        offs = np.cumsum(bs)[:-1]
        pos = (cblk * cfg.tb * 128) + (np.arange(len(csrc)) - offs[cblk])
        S1[pos] = csrc
        D1[pos] = cdst
        S2[pos] = remap[csrc]
        D2[pos] = remap[cdst]
        DL[pos] = (cloc % 128).astype(np.float32)

        def tposed(a):
            return np.ascontiguousarray(a.reshape(cfg.nt, 128).T)

        # one-hot S: SH[p, t*128+d] = (DL[t*128+p] == d), fp8 (exact 0/1)
        import ml_dtypes
        dlt = tposed(DL)  # [128, nt]
        sh_ = (dlt[:, :, None] == np.arange(128, dtype=np.float32)[None, None, :])
        SH = sh_.reshape(128, cfg.nt * 128).astype(ml_dtypes.float8_e4m3)

        pid = np.full(cfg.sh, -1.0, np.float32)
        pw = np.zeros(cfg.sh, np.float32)
        lo, hi = k * per, (k + 1) * per
        pid[:per] = batch[lo:hi].astype(np.float32)
        pw[:per] = wg[batch[lo:hi]]
        in_maps.append({
            "XT": xt, "W1cat": w1c, "W2cat": w2c,
            "B1M": b1m, "B2M": b2m,
            "WL": np.asarray(Wl, np.float32),
            "BLC": np.asarray(bl, np.float32).reshape(OUT, 1),
            "SRC1": tposed(S1), "DST1": tposed(D1),
            "SRC2": tposed(S2), "DST2": tposed(D2),
            "SH": SH,
            "PID": np.ascontiguousarray(pid.reshape(nb, 128).T),
            "PW": np.ascontiguousarray(pw.reshape(nb, 128).T),
        })
    return cfg, in_maps


def build(cfg: Cfg):
    nc = bacc.Bacc("TRN2", target_bir_lowering=False, debug=False,
                   num_devices=cfg.ncores)
    NB, TB, NT = cfg.nb, cfg.tb, cfg.nt
    NP1 = cfg.npad // 128
    NP2 = cfg.n2 // 128

    ein = lambda nm, sh, dt: nc.dram_tensor(nm, sh, dt, kind="ExternalInput").ap()
    XT = ein("XT", [FEAT, cfg.npad], F32)
    W1c = ein("W1cat", [FEAT, 136], F32)
    W2c = ein("W2cat", [FEAT, 136], F32)
    B1M = ein("B1M", [128, FEAT], F32)
    B2M = ein("B2M", [128, FEAT], F32)
    WL = ein("WL", [FEAT, OUT], F32)
    BLC = ein("BLC", [OUT, 1], F32)
    SRC1 = ein("SRC1", [128, NT], I32)
    DST1 = ein("DST1", [128, NT], I32)
    SRC2 = ein("SRC2", [128, NT], I32)
    DST2 = ein("DST2", [128, NT], I32)
    F8 = mybir.dt.float8e4
    SH = ein("SH", [128, NT * 128], F8)
    PID = ein("PID", [128, NB], F32)
    PW = ein("PW", [128, NB], F32)
    OUTT = nc.dram_tensor("OUT", [64, OUT], F32, kind="ExternalOutput").ap()

    H1p = nc.dram_tensor("H1p", [cfg.npad, ROWB], U8).ap()
    h1sh = nc.dram_tensor("h1sh", [cfg.sh, FEAT], BF16).ap()
    h1full = nc.dram_tensor("h1full", [cfg.n2, FEAT], BF16,
                            addr_space="Shared").ap()
    H2p = nc.dram_tensor("H2p", [cfg.n2, ROWB], U8).ap()
    prt = nc.dram_tensor("prt", [FEAT, 64], F32).ap()
    prf = nc.dram_tensor("prf", [FEAT, 64], F32, addr_space="Shared").ap()

    groups = [list(range(cfg.ncores))]

    with tile.TileContext(nc) as tc:
        import contextlib
        ctx = contextlib.ExitStack()
        with ctx:
            consts = ctx.enter_context(tc.tile_pool(name="consts", bufs=1))
            xtp = ctx.enter_context(tc.tile_pool(name="xtp", bufs=6))
            hp = ctx.enter_context(tc.tile_pool(name="hp", bufs=6))
            hps = ctx.enter_context(tc.tile_pool(name="hps", bufs=4, space="PSUM"))
            gp = ctx.enter_context(tc.tile_pool(name="gp", bufs=4))
            sp = ctx.enter_context(tc.tile_pool(name="sp", bufs=4))
            up = ctx.enter_context(tc.tile_pool(name="up", bufs=3, space="PSUM"))
            ep = ctx.enter_context(tc.tile_pool(name="ep", bufs=4))
            pp = ctx.enter_context(tc.tile_pool(name="pp", bufs=1, space="PSUM"))
            tp = ctx.enter_context(tc.tile_pool(name="tp", bufs=1))

            w1sb = consts.tile([FEAT, 136], F32, tag="w1")
            nc.sync.dma_start(out=w1sb[:], in_=W1c)
            w1sb_b = consts.tile([FEAT, 136], BF16, tag="w1b")
            nc.vector.tensor_copy(out=w1sb_b[:], in_=w1sb[:])
            w2sb_f = consts.tile([FEAT, 136], F32, tag="w2f")
            nc.sync.dma_start(out=w2sb_f[:], in_=W2c)
            w2sb = consts.tile([FEAT, 136], BF16, tag="w2b")
            nc.vector.tensor_copy(out=w2sb[:], in_=w2sb_f[:])
            b1sb = consts.tile([128, FEAT], F32, tag="b1")
            nc.sync.dma_start(out=b1sb[:], in_=B1M)
            b2sb = consts.tile([128, FEAT], F32, tag="b2")
            nc.sync.dma_start(out=b2sb[:], in_=B2M)
            wlsb = consts.tile([FEAT, OUT], F32, tag="wl")
            nc.sync.dma_start(out=wlsb[:], in_=WL)
            blsb = consts.tile([OUT, 1], F32, tag="bl")
            nc.sync.dma_start(out=blsb[:], in_=BLC)
            ident = consts.tile([128, 128], F32, tag="ident")
            make_identity(nc, ident[:])
            iotaI = consts.tile([128, 128], I32, tag="iotai")
            nc.gpsimd.iota(iotaI[:], pattern=[[1, 128]], base=0,
                           channel_multiplier=0)
            iotaB = consts.tile([128, 128], BF16, tag="iotab")
            nc.vector.tensor_copy(out=iotaB[:], in_=iotaI[:])
            iota64 = consts.tile([128, 64], F32, tag="iota64")
            nc.vector.tensor_copy(out=iota64[:], in_=iotaI[:, :64])
            pidsb = consts.tile([128, NB], F32, tag="pid")
            nc.sync.dma_start(out=pidsb[:], in_=PID)
            pwsb = consts.tile([128, NB], F32, tag="pw")
            nc.sync.dma_start(out=pwsb[:], in_=PW)

            idxp = ctx.enter_context(tc.tile_pool(name="idxp", bufs=1))
            src1 = idxp.tile([128, NT], I32, tag="src1")
            nc.sync.dma_start(out=src1[:], in_=SRC1)
            dst1 = idxp.tile([128, NT], I32, tag="dst1")
            nc.sync.dma_start(out=dst1[:], in_=DST1)
            src2 = idxp.tile([128, NT], I32, tag="src2")
            nc.sync.dma_start(out=src2[:], in_=SRC2)
            dst2 = idxp.tile([128, NT], I32, tag="dst2")
            nc.sync.dma_start(out=dst2[:], in_=DST2)

            def h_phase(nblocks, lhs_group, wsb, Hp):
                grp = cfg.xt_grp
                for m0 in range(0, nblocks, grp):
                    gi = m0 // grp
                    mcnt = min(grp, nblocks - m0)
                    lhs_of = lhs_group(m0, mcnt)
                    sb = hp.tile([128, grp * ROWB], U8, tag="hrow")
                    sbB = sb[:].bitcast(BF16)   # 144 bf16 per row
                    sbF = sb[:].bitcast(F32)    # 72 f32 per row
                    for j in range(mcnt):
                        ps = hps.tile([128, 136], F32, tag="hpsum")
                        nc.tensor.matmul(ps[:], lhsT=lhs_of(j), rhs=wsb[:],
                                         start=True, stop=True)
                        if j % 2 == 0:
                            nc.scalar.copy(out=sbB[:, j * 144:j * 144 + FEAT],
                                           in_=ps[:, :FEAT])
                        else:
                            nc.vector.tensor_copy(
                                out=sbB[:, j * 144:j * 144 + FEAT],
                                in_=ps[:, :FEAT])
                        nc.vector.tensor_copy(
                            out=sbF[:, j * 72 + 64:j * 72 + 72],
                            in_=ps[:, FEAT:FEAT + 2 * HEADS])
                    ho = Hp[m0 * 128:(m0 + mcnt) * 128, :].rearrange(
                        "(b p) c -> p b c", p=128)
                    weng = nc.scalar if gi % 2 == 0 else nc.sync
                    weng.dma_start(
                        out=ho, in_=sb[:, :mcnt * ROWB].rearrange(
                            "p (b c) -> p b c", c=ROWB))

            def p1_lhs(m0, mcnt):
                t = xtp.tile([128, cfg.xt_grp * 128], BF16, tag="xt")
                nc.gpsimd.dma_start(out=t[:, :mcnt * 128],
                                    in_=XT[:, m0 * 128:(m0 + mcnt) * 128])
                return lambda j: t[:, j * 128:(j + 1) * 128]

            h_phase(NP1, p1_lhs, w1sb_b, H1p)

            def edge_phase(srcT, dstT, Hp, layer):
                hgrp = cfg.h_grp
                h1grp = None
                g1 = g2 = None
                PAIR = 1  # blocks per gather instruction
                for b in range(NB):
                    if b % PAIR == 0:
                        bw = min(PAIR, NB - b)  # blocks in this gather
                        t0g = b * TB
                        g1 = gp.tile([128, PAIR * TB * ROWB], U8, tag="g1")
                        nc.gpsimd.indirect_dma_start(
                            out=g1[:, :bw * TB * ROWB], out_offset=None,
                            in_=Hp,
                            in_offset=bass.IndirectOffsetOnAxis(
                                ap=srcT[:, t0g:t0g + bw * TB], axis=0))
                        g2 = gp.tile([128, PAIR * TB * 4 * HEADS], U8, tag="g2")
                        nc.gpsimd.indirect_dma_start(
                            out=g2[:, :bw * TB * 4 * HEADS], out_offset=None,
                            in_=Hp,
                            in_offset=bass.IndirectOffsetOnAxis(
                                ap=dstT[:, t0g:t0g + bw * TB], axis=0),
                            element_offset=256 + 4 * HEADS)
                    sub = b % PAIR
                    t0 = b * TB
                    g1B = g1[:].bitcast(BF16)[
                        :, sub * TB * 144:(sub + 1) * TB * 144]
                    g1F = g1[:].bitcast(F32)[
                        :, sub * TB * 72:(sub + 1) * TB * 72]
                    g2s = g2[:].bitcast(F32)[
                        :, sub * TB * HEADS:(sub + 1) * TB * HEADS]
                    lg = sp.tile([128, TB * HEADS], F32, tag="lg")
                    als = g1F.rearrange("p (t c) -> p t c", c=72)[:, :, 64:68]
                    nc.vector.tensor_tensor(
                        out=lg[:].rearrange("p (t c) -> p t c", c=HEADS),
                        in0=als,
                        in1=g2s.rearrange("p (t c) -> p t c", c=HEADS),
                        op=mybir.AluOpType.add)
                    lg2 = sp.tile([128, TB * HEADS], F32, tag="lg2")
                    nc.vector.tensor_scalar_mul(lg2[:], lg[:], NEG_SLOPE)
                    nc.vector.tensor_tensor(out=lg[:], in0=lg[:], in1=lg2[:],
                                            op=mybir.AluOpType.max)
                    pv = g1B.rearrange("p (t c) -> p t c", c=144)[
                        :, :, FEAT:FEAT + HEADS]
                    nc.scalar.activation(
                        out=pv,
                        in_=lg[:].rearrange("p (t c) -> p t c", c=HEADS),
                        func=mybir.ActivationFunctionType.Exp)
                    # p expanded densely on ScalarE so the big VectorE
                    # tensor_tensor runs in the 2x perf mode
                    pX = sp.tile([128, TB * 128], BF16, tag="pX")
                    nc.scalar.copy(
                        out=pX[:].rearrange("p (t h c) -> p t h c",
                                            h=HEADS, c=HID),
                        in_=g1B.rearrange("p (t c) -> p t c", c=144)[
                            :, :, FEAT:FEAT + HEADS].unsqueeze(3).to_broadcast(
                            [128, TB, HEADS, HID]))
                    S = sp.tile([128, TB * 128], F8, tag="S")
                    nc.sync.dma_start(
                        out=S[:], in_=SH[:, t0 * 128:(t0 + TB) * 128])
                    hv = g1B.rearrange("p (t c) -> p t c", c=144)[:, :, :FEAT]
                    nc.vector.tensor_tensor(
                        out=hv, in0=hv,
                        in1=pX[:].rearrange("p (t c) -> p t c", c=128),
                        op=mybir.AluOpType.mult)
                    ups = up.tile([128, FEAT + HEADS], F32, tag="u")
                    for t in range(TB):
                        nc.tensor.matmul(
                            ups[:],
                            lhsT=S[:, t * 128:(t + 1) * 128],
                            rhs=g1B[:, t * 144:t * 144 + FEAT + HEADS],
                            start=(t == 0), stop=(t == TB - 1))
                    z = ep.tile([128, HEADS], F32, tag="z")
                    nc.vector.tensor_scalar_add(z[:], ups[:, FEAT:FEAT + HEADS],
                                                EPS)
                    rz = ep.tile([128, HEADS], F32, tag="rz")
                    nc.vector.reciprocal(rz[:], z[:])
                    o1 = ep.tile([128, FEAT], F32, tag="o1")
                    nc.vector.tensor_tensor(
                        out=o1[:].rearrange("p (h c) -> p h c", c=HID),
                        in0=ups[:, :FEAT].rearrange("p (h c) -> p h c", c=HID),
                        in1=rz[:].unsqueeze(2).to_broadcast([128, HEADS, HID]),
                        op=mybir.AluOpType.mult)
                    if layer == 1:
                        nc.vector.tensor_tensor(out=o1[:], in0=o1[:],
                                                in1=b1sb[:],
                                                op=mybir.AluOpType.add)
                        mn = ep.tile([128, FEAT], F32, tag="mn")
                        nc.vector.tensor_scalar_min(mn[:], o1[:], 0.0)
                        ex = ep.tile([128, FEAT], F32, tag="ex")
                        nc.scalar.activation(
                            out=ex[:], in_=mn[:],
                            func=mybir.ActivationFunctionType.Exp)
                        nc.vector.tensor_scalar(
                            o1[:], o1[:], 0.0, -1.0,
                            op0=mybir.AluOpType.max, op1=mybir.AluOpType.add)
                        if h1grp is None or b % hgrp == 0:
                            h1grp = ep.tile([128, hgrp * FEAT], BF16, tag="h1g")
                        nc.vector.tensor_tensor(
                            out=h1grp[:, (b % hgrp) * FEAT:
                                      (b % hgrp + 1) * FEAT],
                            in0=o1[:], in1=ex[:], op=mybir.AluOpType.add)
                        if b % hgrp == hgrp - 1 or b == NB - 1:
                            blo = (b // hgrp) * hgrp
                            bcnt = b - blo + 1
                            ho = h1sh[blo * 128:(b + 1) * 128, :].rearrange(
                                "(q p) c -> p q c", p=128)
                            nc.scalar.dma_start(
                                out=ho,
                                in_=h1grp[:, :bcnt * FEAT].rearrange(
                                    "p (q c) -> p q c", c=FEAT))
                    else:
                        h2 = ep.tile([128, FEAT], F32, tag="h2")
                        nc.vector.tensor_tensor(out=h2[:], in0=o1[:],
                                                in1=b2sb[:],
                                                op=mybir.AluOpType.add)
                        nc.tensor.matmul(
                            ppsum[:], lhsT=h2[:],
                            rhs=spgall[:, b * 64:(b + 1) * 64],
                            start=(b == 0), stop=(b == NB - 1),
                            skip_group_check=True)

            edge_phase(src1, dst1, H1p, layer=1)

            nc.gpsimd.collective_compute(
                "AllGather", mybir.AluOpType.bypass, replica_groups=groups,
                ins=[h1sh.opt()], outs=[h1full.opt()])

            def p3_lhs(m0, mcnt):
                t = xtp.tile([128, cfg.xt_grp * 128], BF16, tag="h1t")
                nc.sync.dma_start(
                    out=t[:, :mcnt * 128],
                    in_=h1full[m0 * 128:(m0 + mcnt) * 128, :],
                    transpose=True)
                return lambda j: t[:, j * 128:(j + 1) * 128]

            h_phase(NP2, p3_lhs, w2sb, H2p)

            spgall = idxp.tile([128, NB * 64], F32, tag="spgall")
            nc.vector.tensor_tensor(
                out=spgall[:].rearrange("p (b c) -> p b c", c=64),
                in0=pidsb[:].unsqueeze(2).to_broadcast([128, NB, 64]),
                in1=iota64[:].unsqueeze(1).to_broadcast([128, NB, 64]),
                op=mybir.AluOpType.is_equal)
            nc.vector.tensor_tensor(
                out=spgall[:].rearrange("p (b c) -> p b c", c=64),
                in0=spgall[:].rearrange("p (b c) -> p b c", c=64),
                in1=pwsb[:].unsqueeze(2).to_broadcast([128, NB, 64]),
                op=mybir.AluOpType.mult)
            ppsum = pp.tile([128, 64], F32, tag="pool")
            edge_phase(src2, dst2, H2p, layer=2)

            psb = tp.tile([128, 64], F32, tag="psb")
            nc.vector.tensor_copy(out=psb[:], in_=ppsum[:])
            nc.sync.dma_start(out=prt, in_=psb[:])
            nc.gpsimd.collective_compute(
                "AllReduce", mybir.AluOpType.add, replica_groups=groups,
                ins=[prt.opt()], outs=[prf.opt()])
            pall = tp.tile([128, 64], F32, tag="pall")
            nc.sync.dma_start(out=pall[:], in_=prf)
            lps = up.tile([OUT, 64], F32, tag="u")
            nc.tensor.matmul(lps[:], lhsT=wlsb[:], rhs=pall[:],
                             start=True, stop=True)
            lsb = tp.tile([OUT, 64], F32, tag="lsb")
            nc.vector.tensor_scalar(lsb[:], lps[:], blsb[:, :1], None,
                                    op0=mybir.AluOpType.add)
            tps = up.tile([64, OUT], F32, tag="u")
            nc.tensor.transpose(out=tps[:], in_=lsb[:],
                                identity=ident[:OUT, :OUT])
            sm = tp.tile([64, OUT], F32, tag="sm")
            nc.vector.tensor_copy(out=sm[:], in_=tps[:])
            mx = tp.tile([64, 1], F32, tag="mx")
            nc.vector.reduce_max(mx[:], sm[:], axis=mybir.AxisListType.X)
            nc.vector.tensor_scalar(sm[:], sm[:], mx[:, :1], None,
                                    op0=mybir.AluOpType.subtract)
            nc.scalar.activation(out=sm[:], in_=sm[:],
                                 func=mybir.ActivationFunctionType.Exp)
            ssum = tp.tile([64, 1], F32, tag="ssum")
            nc.vector.reduce_sum(ssum[:], sm[:], axis=mybir.AxisListType.X)
            rs = tp.tile([64, 1], F32, tag="rs")
            nc.vector.reciprocal(rs[:], ssum[:])
            nc.vector.tensor_scalar(sm[:], sm[:], rs[:, :1], None,
                                    op0=mybir.AluOpType.mult)
            nc.sync.dma_start(out=OUTT, in_=sm[:])

    nc.compile()
    return nc


_CACHE = {}


def kernel(**inputs) -> np.ndarray:
    ncores = 8
    cfg, in_maps = host_prep(ncores=ncores, **inputs)
    key = dataclasses.astuple(cfg)
    if key not in _CACHE:
        _CACHE[key] = build(cfg)
    nc = _CACHE[key]
    res = bass_utils.run_bass_kernel_spmd(nc, in_maps,
                                          core_ids=list(range(ncores)))
    out = res.results[0]["OUT"][:64]
    return np.asarray(out, np.float32)


if __name__ == "__main__":
    # quick self-run with random data matching the spec
    rng = np.random.default_rng(0)
    ins = {
        "x": rng.standard_normal((50000, 128), np.float32),
        "W1": (rng.standard_normal((128, 128)) * 0.05).astype(np.float32),
        "a1_src": (rng.standard_normal((4, 32)) * 0.05).astype(np.float32),
        "a1_dst": (rng.standard_normal((4, 32)) * 0.05).astype(np.float32),
        "b1": np.zeros(128, np.float32),
        "W2": (rng.standard_normal((128, 128)) * 0.05).astype(np.float32),
        "a2_src": (rng.standard_normal((4, 32)) * 0.05).astype(np.float32),
        "a2_dst": (rng.standard_normal((4, 32)) * 0.05).astype(np.float32),
        "b2": np.zeros(128, np.float32),
        "Wl": (rng.standard_normal((128, 10)) * 0.05).astype(np.float32),
        "bl": np.zeros(10, np.float32),
        "edge_index": rng.integers(0, 50000, (2, 800000)).astype(np.int32),
        "batch": np.sort(rng.integers(0, 64, 50000)).astype(np.int32),
    }
    out = kernel(**ins)
    print(out.shape, out.dtype, out[:2])
